# revision 1
# baseline (speedup 1.0000x reference)
"""Multi-head causal attention (B=4,T=2048,E=1024,H=16,D=64) on 8 TRN2 cores.

Sharding: core c -> batch b=c//2, heads h0=(c%2)*8 .. h0+8 (tensor-parallel
over head halves within each batch). Each core computes its 8 heads'
attention and a partial output projection (row-split Wp); host sums the
two partials per batch (+bias).

Per-core kernel (all matmuls bf16 x bf16 -> fp32 PSUM):
  qT/kT = W.T @ x.T          [64*2, T] per head-pair (full PE efficiency)
  v     = x @ Wv             [T, 64] per head, +ones column for softmax sum
  sT    = kT.T @ qT          [tk, tq] blocks (K=64)
  expT  = exp(sT/8) (ACT), causal mask multiply on diagonal blocks (DVE)
  oT,Z  = [v|1].T @ expT     accumulated over tk chunks; row 64 = sum(exp)
  oT   /= Z  (DVE recip + gpsimd partition_broadcast + DVE mul)
  part  = oT.T @ WpT         [T, E] partial projection, fp32 out
"""
import sys
import numpy as np

sys.path.insert(0, "/opt/trn_rl_repo")

import ml_dtypes
import concourse.bass as bass
import concourse.bacc as bacc
import concourse.mybir as mybir
from concourse import tile
from concourse.bass_utils import run_bass_kernel_spmd

B, T, E, H, D = 4, 2048, 1024, 16, 64
HL = H // 2      # 8 local heads per core
NP = HL // 2     # 4 head pairs
NJ = T // 512    # 4 tq tiles
NCK = T // 128   # 16 t chunks
NE = E // 128    # 8 e chunks
BF16 = mybir.dt.bfloat16
F32 = mybir.dt.float32
bfnp = ml_dtypes.bfloat16

_CACHE = {}


def _build():
    nc = bacc.Bacc("TRN2", target_bir_lowering=False)
    xT = nc.declare_dram_parameter("xT", [E, T], BF16, isOutput=False)
    wq = nc.declare_dram_parameter("wq", [E, HL * D], BF16, isOutput=False)
    wk = nc.declare_dram_parameter("wk", [E, HL * D], BF16, isOutput=False)
    wv = nc.declare_dram_parameter("wv", [E, HL * D], BF16, isOutput=False)
    wpT = nc.declare_dram_parameter("wpT", [HL * D, E], BF16, isOutput=False)
    maskb = nc.declare_dram_parameter("maskb", [128, 512], F32, isOutput=False)
    out = nc.declare_dram_parameter("out", [T, E], F32, isOutput=True)

    Exp = mybir.ActivationFunctionType.Exp

    with tile.TileContext(nc) as tc:
        with (
            tc.tile_pool(name="persist", bufs=1) as pp,
            tc.tile_pool(name="expp", bufs=12) as expp,
            tc.tile_pool(name="zpool", bufs=4) as zpool,
            tc.tile_pool(name="outp", bufs=4) as outp,
            tc.tile_pool(name="mm", bufs=2, space=bass.MemorySpace.PSUM) as ps_mm,
            tc.tile_pool(name="sT", bufs=3, space=bass.MemorySpace.PSUM) as ps_sT,
            tc.tile_pool(name="oT", bufs=3, space=bass.MemorySpace.PSUM) as ps_oT,
        ):
            xT_sb = [pp.tile([128, T], BF16, tag=f"xT{c}", name=f"xT{c}") for c in range(NE)]
            wv_sb = [pp.tile([128, HL * D], BF16, tag=f"wv{c}", name=f"wv{c}") for c in range(NE)]
            wq_sb = [pp.tile([128, HL * D], BF16, tag=f"wq{c}", name=f"wq{c}") for c in range(NE)]
            wk_sb = [pp.tile([128, HL * D], BF16, tag=f"wk{c}", name=f"wk{c}") for c in range(NE)]
            wpT_sb = [pp.tile([128, E], BF16, tag=f"wp{p}", name=f"wp{p}") for p in range(NP)]
            mask_sb = pp.tile([128, 512], F32, tag="mkb", name="mkb")
            v_sb = [pp.tile([128, HL * 65], BF16, tag=f"v{i}", name=f"v{i}") for i in range(NCK)]
            qT_sb = [[pp.tile([128, 512], BF16, tag=f"q{p}_{j}", name=f"q{p}_{j}") for j in range(NJ)]
                     for p in range(NP)]
            kT_sb = [[pp.tile([128, 512], BF16, tag=f"k{p}_{j}", name=f"k{p}_{j}") for j in range(NJ)]
                     for p in range(NP)]
            oT_sb = [[pp.tile([128, 512], BF16, tag=f"o{p}_{j}", name=f"o{p}_{j}") for j in range(NJ)]
                     for p in range(NP)]

            for c in range(NE):
                nc.sync.dma_start(wv_sb[c][:], wv[c * 128:(c + 1) * 128, :])
                nc.sync.dma_start(xT_sb[c][:, 0:512], xT[c * 128:(c + 1) * 128, 0:512])
            for q in range(1, 4):
                for c in range(NE):
                    nc.sync.dma_start(xT_sb[c][:, q * 512:(q + 1) * 512],
                                      xT[c * 128:(c + 1) * 128, q * 512:(q + 1) * 512])
            for c in range(NE):
                nc.sync.dma_start(wq_sb[c][:], wq[c * 128:(c + 1) * 128, :])
                nc.sync.dma_start(wk_sb[c][:], wk[c * 128:(c + 1) * 128, :])
            for p in range(NP):
                nc.sync.dma_start(wpT_sb[p][:], wpT[p * 128:(p + 1) * 128, :])
            nc.sync.dma_start(mask_sb[:], maskb[:])

            # V projection per t-chunk, interleaved [v_h | 1] layout
            for i in range(NCK):
                ps = ps_mm.tile([128, HL * D], F32, tag="mm", name="mmv")
                for c in range(NE):
                    nc.tensor.matmul(ps[:], xT_sb[c][:, i * 128:(i + 1) * 128],
                                     wv_sb[c][:], start=(c == 0), stop=(c == NE - 1))
                v3 = v_sb[i][:].rearrange("p (h d) -> p h d", d=65)
                nc.gpsimd.memset(v_sb[i][:], 1.0)
                nc.vector.tensor_copy(
                    v3[:, :, 0:64], ps[:].rearrange("p (h d) -> p h d", d=64))

            # Q/K projections: per head-pair p, tq tile j -> qT/kT [128, 512]
            for j in range(NJ):
                for p in range(NP):
                    ps = ps_mm.tile([128, 512], F32, tag="mm", name="mmq")
                    for c in range(NE):
                        nc.tensor.matmul(ps[:], wq_sb[c][:, p * 128:(p + 1) * 128],
                                         xT_sb[c][:, j * 512:(j + 1) * 512],
                                         start=(c == 0), stop=(c == NE - 1))
                    nc.vector.tensor_copy(qT_sb[p][j][:], ps[:])
                    ps2 = ps_mm.tile([128, 512], F32, tag="mm", name="mmq")
                    for c in range(NE):
                        nc.tensor.matmul(ps2[:], wk_sb[c][:, p * 128:(p + 1) * 128],
                                         xT_sb[c][:, j * 512:(j + 1) * 512],
                                         start=(c == 0), stop=(c == NE - 1))
                    nc.vector.tensor_copy(kT_sb[p][j][:], ps2[:])

            # attention per (tq tile j, head h); diagonal chunks narrowed to
            # the unmasked column range, mask multiply only on the 128-wide
            # triangle region after exp
            for j in range(NJ):
                for h in range(HL):
                    p, r0 = h // 2, (h % 2) * 64
                    oT_ps = ps_oT.tile([65, 512], F32, tag="oT", name="oTps")
                    nblk = 4 * (j + 1)
                    for c in range(nblk):
                        r = max(0, (c - 4 * j) * 128)
                        w = 512 - r
                        sT = ps_sT.tile([128, 512], F32, tag="sT", name="sTps")
                        nc.tensor.matmul(
                            sT[:, 0:w],
                            kT_sb[p][c // 4][r0:r0 + 64,
                                             (c % 4) * 128:(c % 4 + 1) * 128],
                            qT_sb[p][j][r0:r0 + 64, r:512],
                            start=True, stop=True)
                        et = expp.tile([128, 512], BF16, tag="expT", name="et")
                        nc.scalar.activation(et[:, 0:w], sT[:, 0:w], Exp, scale=0.125)
                        if c >= 4 * j:
                            mw = min(128, w)
                            nc.vector.tensor_mul(et[:, 0:mw], et[:, 0:mw],
                                                 mask_sb[:, 0:mw])
                        nc.tensor.matmul(oT_ps[:, r:512],
                                         v_sb[c][:, h * 65:(h + 1) * 65],
                                         et[:, 0:w],
                                         start=(c == 0), stop=(c == nblk - 1),
                                         skip_group_check=True)
                    zi = zpool.tile([1, 512], F32, tag="zi", name="zi")
                    nc.vector.reciprocal(zi[:], oT_ps[64:65, :])
                    zb = zpool.tile([64, 512], F32, tag="zb", name="zb")
                    nc.gpsimd.partition_broadcast(zb[:], zi[:])
                    nc.vector.tensor_mul(oT_sb[p][j][r0:r0 + 64, :],
                                         oT_ps[0:64, :], zb[:])

                for t in range(4 * j, 4 * j + 4):
                    ob = outp.tile([128, E], F32, tag="ob", name="ob")
                    for n in range(2):
                        ps = ps_mm.tile([128, 512], F32, tag="mm", name="mmq")
                        for p in range(NP):
                            nc.tensor.matmul(
                                ps[:],
                                oT_sb[p][j][:, (t % 4) * 128:(t % 4 + 1) * 128],
                                wpT_sb[p][:, n * 512:(n + 1) * 512],
                                start=(p == 0), stop=(p == NP - 1))
                        nc.vector.tensor_copy(ob[:, n * 512:(n + 1) * 512], ps[:])
                    nc.sync.dma_start(out[t * 128:(t + 1) * 128, :], ob[:])

    nc.compile()
    return nc


def _masks_np():
    f = np.arange(512)[None, :]
    p = np.arange(128)[:, None]
    return (f >= p).astype(np.float32)


def kernel(x, Wq, Wk, Wv, Wp, bp):
    x = np.asarray(x, dtype=np.float32)
    Wq = np.asarray(Wq, dtype=np.float32)
    Wk = np.asarray(Wk, dtype=np.float32)
    Wv = np.asarray(Wv, dtype=np.float32)
    Wp = np.asarray(Wp, dtype=np.float32)
    bp = np.asarray(bp, dtype=np.float32)

    if "nc" not in _CACHE:
        _CACHE["nc"] = _build()
    nc = _CACHE["nc"]

    masks = _masks_np()
    WpT = np.ascontiguousarray(Wp.T).astype(bfnp)  # [E(hd), E(n)]
    xTs = [np.ascontiguousarray(x[b].T).astype(bfnp) for b in range(B)]

    def wslice(W, h0):  # [H,E,D] -> [E, 8*64] col = local head*64+d
        return np.ascontiguousarray(
            W[h0:h0 + HL].transpose(1, 0, 2).reshape(E, HL * D)).astype(bfnp)

    in_maps = []
    for c in range(8):
        b, hh = c // 2, c % 2
        h0 = hh * HL
        in_maps.append({
            "xT": xTs[b],
            "wq": wslice(Wq, h0),
            "wk": wslice(Wk, h0),
            "wv": wslice(Wv, h0),
            "wpT": np.ascontiguousarray(WpT[h0 * D:(h0 + HL) * D, :]),
            "maskb": masks,
        })

    res = run_bass_kernel_spmd(nc, in_maps, list(range(8)))
    parts = [np.asarray(res.results[c]["out"], dtype=np.float32) for c in range(8)]
    out = np.stack([parts[2 * b] + parts[2 * b + 1] for b in range(B)], axis=0)
    return (out + bp[None, None, :]).astype(np.float32)



# revision 7
# speedup vs baseline: 1.0888x; 1.0888x over previous
"""Multi-head causal attention (B=4,T=2048,E=1024,H=16,D=64) on 8 TRN2 cores.

Sharding: core c -> batch b=c//2, heads h0=(c%2)*8 (tensor-parallel over head
halves within each batch). Each core computes its 8 heads' attention and a
partial output projection (row-split Wp); host sums the two partials per
batch (+bias).

v2: fp8e4 DoubleRow matmuls for Q/K projections (K_eff=256/instr, 0.5
cyc/row) and attention scores (d=64 folded as [32,2]); bf16 PV with the
ones-row softmax-sum trick; exp on ACT over 2-bank PSUM score-pair tiles.
Q/K weights are pre-scaled x32 on host (exp scale absorbs /1024).

Per-core kernel:
  v     = x @ Wv                   bf16, [t,hd] + ones column
  q,k   = x8 @ W8 (DoubleRow)      fp8 -> f32 PSUM -> fp8 [32,2,T] fold (DMA)
  sT    = k8.T @ q8 (DoubleRow)    [128 keys, 512 tq] per chunk, pairs share
                                   a [128,2,512] PSUM tile (2 banks)
  expT  = exp(sT*scale) (ACT, one instr per chunk pair), causal mask mul on
          diagonal chunks (DVE bf16)
  oT,Z  = [v|1].T @ expT           bf16, accumulated over chunks; row 64 = Z
  oT   /= Z (DVE recip + gpsimd broadcast + DVE mul)
  part  = oT.T @ WpT               bf16 -> f32 out
"""
import sys
import numpy as np

sys.path.insert(0, "/opt/trn_rl_repo")

import ml_dtypes
import concourse.bass as bass
import concourse.bacc as bacc
import concourse.mybir as mybir
from concourse import tile
from concourse.bass_utils import run_bass_kernel_spmd

B, T, E, H, D = 4, 2048, 1024, 16, 64
HL = H // 2      # 8 local heads per core
NP = HL // 2     # 4 head pairs
NJ = T // 512    # 4 tq tiles
NCK = T // 128   # 16 t chunks
NE = E // 128    # 8 e chunks (bf16)
NE2 = E // 256   # 4 e pair-chunks (fp8 DoubleRow)
BF16 = mybir.dt.bfloat16
F32 = mybir.dt.float32
FP8 = mybir.dt.float8e4
bfnp = ml_dtypes.bfloat16
fp8np = ml_dtypes.float8_e4m3
DR = mybir.MatmulPerfMode.DoubleRow

WSCALE = 32.0
EXPSCALE = 0.125 / (WSCALE * WSCALE)

_CACHE = {}


def _build():
    nc = bacc.Bacc("TRN2", target_bir_lowering=False)
    xT = nc.declare_dram_parameter("xT", [E, T], BF16, isOutput=False)
    xT8 = nc.declare_dram_parameter("xT8", [E, T], FP8, isOutput=False)
    w8q = nc.declare_dram_parameter("w8q", [E, HL * D], FP8, isOutput=False)
    w8k = nc.declare_dram_parameter("w8k", [E, HL * D], FP8, isOutput=False)
    wv = nc.declare_dram_parameter("wv", [E, HL * D], BF16, isOutput=False)
    wpT = nc.declare_dram_parameter("wpT", [HL * D, E], BF16, isOutput=False)
    maskb = nc.declare_dram_parameter("maskb", [128, 128], BF16, isOutput=False)
    out = nc.declare_dram_parameter("out", [T, E], F32, isOutput=True)

    Exp = mybir.ActivationFunctionType.Exp

    with tile.TileContext(nc) as tc:
        with (
            tc.tile_pool(name="persist", bufs=1) as pp,
            tc.tile_pool(name="qtmp", bufs=3) as qtmp,
            tc.tile_pool(name="expp", bufs=6) as expp,
            tc.tile_pool(name="zpool", bufs=4) as zpool,
            tc.tile_pool(name="outp", bufs=3) as outp,
            tc.tile_pool(name="mm", bufs=2, space=bass.MemorySpace.PSUM) as ps_mm,
            tc.tile_pool(name="sT", bufs=2, space=bass.MemorySpace.PSUM) as ps_sT,
            tc.tile_pool(name="oT", bufs=2, space=bass.MemorySpace.PSUM) as ps_oT,
        ):
            xT_sb = [pp.tile([128, T], BF16, tag=f"xT{c}", name=f"xT{c}") for c in range(NE)]
            x8_sb = [pp.tile([128, 2, T], FP8, tag=f"x8{c}", name=f"x8{c}") for c in range(NE2)]
            w8q_sb = [pp.tile([128, 2, 512], FP8, tag=f"wq{c}", name=f"wq{c}") for c in range(NE2)]
            w8k_sb = [pp.tile([128, 2, 512], FP8, tag=f"wk{c}", name=f"wk{c}") for c in range(NE2)]
            wv_sb = [pp.tile([128, HL * D], BF16, tag=f"wv{c}", name=f"wv{c}") for c in range(NE)]
            wpT_sb = [pp.tile([128, E], BF16, tag=f"wp{p}", name=f"wp{p}") for p in range(NP)]
            mask_sb = pp.tile([128, 128], BF16, tag="mkb", name="mkb")
            v_sb = [pp.tile([128, HL * 65], BF16, tag=f"v{i}", name=f"v{i}") for i in range(NCK)]
            # q8/k8: one [128, 2, T] fp8 tile per head pair; head h%2 at
            # partition base 64*(h%2) (SBUF AP bases must be 0/32/64), d
            # split as (dhalf -> free slot i, 32 -> partition).
            q8_sb = [pp.tile([128, 2, T], FP8, tag=f"q8{p}", name=f"q8{p}") for p in range(NP)]
            k8_sb = [pp.tile([128, 2, T], FP8, tag=f"k8{p}", name=f"k8{p}") for p in range(NP)]
            oT_sb = [[pp.tile([128, 512], BF16, tag=f"o{p}_{j}", name=f"o{p}_{j}") for j in range(NJ)]
                     for p in range(NP)]

            def x8_dram(c2, col0, col1):
                return xT8[256 * c2:256 * (c2 + 1), col0:col1].rearrange(
                    "(i p) t -> p i t", i=2)

            def w8_dram(wsrc, c2):
                return wsrc[256 * c2:256 * (c2 + 1), :].rearrange(
                    "(i p) d -> p i d", i=2)

            # wave 0 + weights
            for c in range(NE):
                nc.sync.dma_start(wv_sb[c][:], wv[c * 128:(c + 1) * 128, :])
                nc.sync.dma_start(xT_sb[c][:, 0:512], xT[c * 128:(c + 1) * 128, 0:512])
            for c2 in range(NE2):
                nc.sync.dma_start(w8q_sb[c2][:], w8_dram(w8q, c2))
                nc.sync.dma_start(w8k_sb[c2][:], w8_dram(w8k, c2))
                nc.sync.dma_start(x8_sb[c2][:, :, 0:512], x8_dram(c2, 0, 512))
            nc.sync.dma_start(mask_sb[:], maskb[:])
            for w in range(1, 4):
                for c in range(NE):
                    nc.sync.dma_start(xT_sb[c][:, w * 512:(w + 1) * 512],
                                      xT[c * 128:(c + 1) * 128, w * 512:(w + 1) * 512])
                for c2 in range(NE2):
                    nc.sync.dma_start(x8_sb[c2][:, :, w * 512:(w + 1) * 512],
                                      x8_dram(c2, w * 512, (w + 1) * 512))
            for p in range(NP):
                nc.sync.dma_start(wpT_sb[p][:], wpT[p * 128:(p + 1) * 128, :])

            # V projection per t-chunk, interleaved [v_h | 1] layout
            for i in range(NCK):
                ps = ps_mm.tile([128, HL * D], F32, tag="mm", name="mmv")
                for c in range(NE):
                    nc.tensor.matmul(ps[:], xT_sb[c][:, i * 128:(i + 1) * 128],
                                     wv_sb[c][:], start=(c == 0), stop=(c == NE - 1))
                v3 = v_sb[i][:].rearrange("p (h d) -> p h d", d=65)
                nc.gpsimd.memset(v_sb[i][:], 1.0)
                nc.vector.tensor_copy(
                    v3[:, :, 0:64], ps[:].rearrange("p (h d) -> p h d", d=64))

            # Q/K projections (fp8 DoubleRow) + fold into [32,2] d-layout
            for j in range(NJ):
                for p in range(NP):
                    for w8_sb, dst in ((w8q_sb, q8_sb), (w8k_sb, k8_sb)):
                        ps = ps_mm.tile([128, 512], F32, tag="mm", name="mmq")
                        for c2 in range(NE2):
                            nc.tensor.matmul(
                                ps[:], w8_sb[c2][:, :, p * 128:(p + 1) * 128],
                                x8_sb[c2][:, :, j * 512:(j + 1) * 512],
                                start=(c2 == 0), stop=(c2 == NE2 - 1),
                                perf_mode=DR)
                        tmp = qtmp.tile([128, 512], FP8, tag="qt", name="qt")
                        nc.vector.tensor_copy(tmp[:], ps[:])
                        for hh in range(2):
                            for half in range(2):
                                nc.sync.dma_start(
                                    dst[p][64 * hh:64 * hh + 32,
                                           half, j * 512:(j + 1) * 512],
                                    tmp[hh * 64 + half * 32:hh * 64 + half * 32 + 32, :])

            # attention per (tq tile j, head h)
            for j in range(NJ):
                for h in range(HL):
                    p, hh = h // 2, h % 2
                    rb = 64 * hh
                    oT_ps = ps_oT.tile([65, 512], F32, tag="oT", name="oTps")
                    nblk = 4 * (j + 1)
                    for cp in range(nblk // 2):
                        cw = []
                        for ci in range(2):
                            c = 2 * cp + ci
                            r = max(0, (c - 4 * j) * 128)
                            cw.append((c, r, 512 - r))
                        sT = ps_sT.tile([128, 2, 512], F32, tag="sT", name="sTps")
                        et = expp.tile([128, 2, 512], BF16, tag="expT", name="et")
                        for ci, (c, r, w) in enumerate(cw):
                            nc.tensor.matmul(
                                sT[:, ci, 0:w],
                                k8_sb[p][rb:rb + 32, :, c * 128:(c + 1) * 128],
                                q8_sb[p][rb:rb + 32, :, j * 512 + r:(j + 1) * 512],
                                start=True, stop=True, perf_mode=DR)
                        # exp over the pair tile in one ACT instr
                        w0, w1 = cw[0][2], cw[1][2]
                        if w1 == 512:           # off-diagonal pair
                            nc.scalar.activation(et[:], sT[:], Exp, scale=EXPSCALE)
                        elif w0 == 512:         # diag pair 1: widths 512, 384
                            sflat = sT[:].rearrange("p a b -> p (a b)")
                            eflat = et[:].rearrange("p a b -> p (a b)")
                            nc.scalar.activation(eflat[:, 0:896], sflat[:, 0:896],
                                                 Exp, scale=EXPSCALE)
                        else:                   # diag pair 2: widths 256, 128
                            nc.scalar.activation(et[:, :, 0:256], sT[:, :, 0:256],
                                                 Exp, scale=EXPSCALE)
                        # causal mask on diagonal chunks (first 128 cols of window)
                        for ci, (c, r, w) in enumerate(cw):
                            if c >= 4 * j:
                                nc.vector.tensor_mul(et[:, ci, 0:128],
                                                     et[:, ci, 0:128], mask_sb[:])
                        # PV accumulate
                        v3s = [v_sb[c][:].rearrange("p (h d) -> p h d", d=65)
                               for c, _, _ in cw]
                        for ci, (c, r, w) in enumerate(cw):
                            nc.tensor.matmul(oT_ps[:, r:512],
                                             v3s[ci][:, h, :],
                                             et[:, ci, 0:w],
                                             start=(c == 0), stop=(c == nblk - 1),
                                             skip_group_check=True)
                    zi = zpool.tile([1, 512], F32, tag="zi", name="zi")
                    nc.vector.reciprocal(zi[:], oT_ps[64:65, :])
                    zb = zpool.tile([64, 512], F32, tag="zb", name="zb")
                    nc.gpsimd.partition_broadcast(zb[:], zi[:])
                    nc.vector.tensor_mul(oT_sb[p][j][64 * hh:64 * hh + 64, :],
                                         oT_ps[0:64, :], zb[:])

                # output projection for this j
                for t in range(4 * j, 4 * j + 4):
                    ob = outp.tile([128, E], F32, tag="ob", name="ob")
                    for n in range(2):
                        ps = ps_mm.tile([128, 512], F32, tag="mm", name="mmo")
                        for p in range(NP):
                            nc.tensor.matmul(
                                ps[:],
                                oT_sb[p][j][:, (t % 4) * 128:(t % 4 + 1) * 128],
                                wpT_sb[p][:, n * 512:(n + 1) * 512],
                                start=(p == 0), stop=(p == NP - 1))
                        nc.vector.tensor_copy(ob[:, n * 512:(n + 1) * 512], ps[:])
                    nc.sync.dma_start(out[t * 128:(t + 1) * 128, :], ob[:])

    nc.compile()
    return nc


def _mask_np():
    f = np.arange(128)[None, :]
    p = np.arange(128)[:, None]
    return (f >= p).astype(bfnp)


def kernel(x, Wq, Wk, Wv, Wp, bp):
    x = np.asarray(x, dtype=np.float32)
    Wq = np.asarray(Wq, dtype=np.float32)
    Wk = np.asarray(Wk, dtype=np.float32)
    Wv = np.asarray(Wv, dtype=np.float32)
    Wp = np.asarray(Wp, dtype=np.float32)
    bp = np.asarray(bp, dtype=np.float32)

    if "nc" not in _CACHE:
        _CACHE["nc"] = _build()
    nc = _CACHE["nc"]

    mask = _mask_np()
    WpT = np.ascontiguousarray(Wp.T).astype(bfnp)  # [E(hd), E(n)]
    xTs = [np.ascontiguousarray(x[b].T) for b in range(B)]

    def wslice(W, h0, scale, dt):  # [H,E,D] -> [E, 8*64] col = local head*64+d
        return np.ascontiguousarray(
            (W[h0:h0 + HL] * scale).transpose(1, 0, 2).reshape(E, HL * D)).astype(dt)

    in_maps = []
    for c in range(8):
        b, hhf = c // 2, c % 2
        h0 = hhf * HL
        in_maps.append({
            "xT": xTs[b].astype(bfnp),
            "xT8": xTs[b].astype(fp8np),
            "w8q": wslice(Wq, h0, WSCALE, fp8np),
            "w8k": wslice(Wk, h0, WSCALE, fp8np),
            "wv": wslice(Wv, h0, 1.0, bfnp),
            "wpT": np.ascontiguousarray(WpT[h0 * D:(h0 + HL) * D, :]),
            "maskb": mask,
        })

    res = run_bass_kernel_spmd(nc, in_maps, list(range(8)))
    parts = [np.asarray(res.results[c]["out"], dtype=np.float32) for c in range(8)]
    out = np.stack([parts[2 * b] + parts[2 * b + 1] for b in range(B)], axis=0)
    return (out + bp[None, None, :]).astype(np.float32)


# revision 14
# speedup vs baseline: 1.1468x; 1.0533x over previous
"""Multi-head causal attention (B=4,T=2048,E=1024,H=16,D=64) on 8 TRN2 cores.

Sharding: core c -> batch b=c//2, heads h0=(c%2)*8 (tensor-parallel over head
halves within each batch). Each core computes its 8 heads' attention and a
partial output projection (row-split Wp); host sums the two partials per
batch (+bias).

v3, selective precision:
  - tq tile j=0 (queries 0..511, the largest-|out| rows): fully bf16
    (projection, scores, PV) - zero fp8 noise on early tokens.
  - j>=1: Q/K projections and scores run fp8e4 DoubleRow (2x contraction per
    instr at 0.5 cyc/row); off-diagonal PV runs fp8 DoubleRow over key-chunk
    pairs; diagonal chunk pairs stay bf16 (mask multiply needs exactness).
    Softmax averaging over >=512 keys washes out fp8 quantization noise.
  - exp on ACT, one instruction per chunk-pair over a 2-bank PSUM tile.
  - Q/K fp8 weights pre-scaled x32 (exp scale absorbs /1024); j0 K is
    quantized to fp8 from the bf16 projection, scaled x32 on copy.
Emission interleaves V-proj / next-j projections / prev-j out-proj into the
per-head attention loop so PE has fill work while ACT (the exp bottleneck)
runs.
"""
import sys
import numpy as np

sys.path.insert(0, "/opt/trn_rl_repo")

import ml_dtypes
import concourse.bass as bass
import concourse.bacc as bacc
import concourse.mybir as mybir
from concourse import tile
from concourse.bass_utils import run_bass_kernel_spmd

B, T, E, H, D = 4, 2048, 1024, 16, 64
HL = H // 2      # 8 local heads per core
NP = HL // 2     # 4 head pairs
NJ = T // 512    # 4 tq tiles
NCK = T // 128   # 16 t chunks
NE = E // 128    # 8 e chunks (bf16)
NE2 = E // 256   # 4 e pair-chunks (fp8 DoubleRow)
BF16 = mybir.dt.bfloat16
F32 = mybir.dt.float32
FP8 = mybir.dt.float8e4
bfnp = ml_dtypes.bfloat16
fp8np = ml_dtypes.float8_e4m3
DR = mybir.MatmulPerfMode.DoubleRow
MUL = mybir.AluOpType.mult

WSCALE = 32.0
EXPSCALE8 = 0.125 / (WSCALE * WSCALE)
EXPSCALE = 0.125

_CACHE = {}


def _build():
    nc = bacc.Bacc("TRN2", target_bir_lowering=False)
    xT = nc.declare_dram_parameter("xT", [E, T], BF16, isOutput=False)
    xT8 = nc.declare_dram_parameter("xT8", [E, T], FP8, isOutput=False)
    wq = nc.declare_dram_parameter("wq", [E, HL * D], BF16, isOutput=False)
    wk = nc.declare_dram_parameter("wk", [E, HL * D], BF16, isOutput=False)
    w8q = nc.declare_dram_parameter("w8q", [E, HL * D], FP8, isOutput=False)
    w8k = nc.declare_dram_parameter("w8k", [E, HL * D], FP8, isOutput=False)
    wv = nc.declare_dram_parameter("wv", [E, HL * D], BF16, isOutput=False)
    wpT = nc.declare_dram_parameter("wpT", [HL * D, E], BF16, isOutput=False)
    maskb = nc.declare_dram_parameter("maskb", [128, 128], BF16, isOutput=False)
    out = nc.declare_dram_parameter("out", [T, E], F32, isOutput=True)

    Exp = mybir.ActivationFunctionType.Exp

    with tile.TileContext(nc) as tc:
        with (
            tc.tile_pool(name="persist", bufs=1) as pp,
            tc.tile_pool(name="qtmp", bufs=3) as qtmp,
            tc.tile_pool(name="expp", bufs=4) as expp,
            tc.tile_pool(name="expp8", bufs=4) as expp8,
            tc.tile_pool(name="zpool", bufs=4) as zpool,
            tc.tile_pool(name="outp", bufs=2) as outp,
            tc.tile_pool(name="mm", bufs=2, space=bass.MemorySpace.PSUM) as ps_mm,
            tc.tile_pool(name="sT", bufs=2, space=bass.MemorySpace.PSUM) as ps_sT,
            tc.tile_pool(name="oT", bufs=2, space=bass.MemorySpace.PSUM) as ps_oT,
        ):
            xT_sb = [pp.tile([128, T], BF16, tag=f"xT{c}", name=f"xT{c}") for c in range(NE)]
            x8_sb = [pp.tile([128, 2, T - 512], FP8, tag=f"x8{c}", name=f"x8{c}")
                     for c in range(NE2)]
            wq_sb = [pp.tile([128, HL * D], BF16, tag=f"bq{c}", name=f"bq{c}") for c in range(NE)]
            wk_sb = [pp.tile([128, HL * D], BF16, tag=f"bk{c}", name=f"bk{c}") for c in range(NE)]
            w8q_sb = [pp.tile([128, 2, 512], FP8, tag=f"wq{c}", name=f"wq{c}") for c in range(NE2)]
            w8k_sb = [pp.tile([128, 2, 512], FP8, tag=f"wk{c}", name=f"wk{c}") for c in range(NE2)]
            wv_sb = [pp.tile([128, HL * D], BF16, tag=f"wv{c}", name=f"wv{c}") for c in range(NE)]
            wpT_sb = [pp.tile([128, E], BF16, tag=f"wp{p}", name=f"wp{p}") for p in range(NP)]
            mask_sb = pp.tile([128, 128], BF16, tag="mkb", name="mkb")
            v_sb = [pp.tile([128, HL * 65], BF16, tag=f"v{i}", name=f"v{i}") for i in range(NCK)]
            # v8: [128, HL, 2, 128] fp8 - per-head [2,128] block so the
            # DoubleRow Ldweights pair stride is 128 (s3_lw_dual_fp8 wants
            # an aligned pair stride); v at cols 0:64, ones at col 64.
            v8_sb = [pp.tile([128, HL, 2, 128], FP8, tag=f"v8{i}", name=f"v8{i}")
                     for i in range(6)]
            # j0 bf16 q/k: [128, 512]; rows = head (h%2)*64 + d
            qb_sb = [pp.tile([128, 512], BF16, tag=f"qb{p}", name=f"qb{p}") for p in range(NP)]
            kb_sb = [pp.tile([128, 512], BF16, tag=f"kb{p}", name=f"kb{p}") for p in range(NP)]
            # fp8 q/k: [128, 2, T]; head h%2 at partition base 64*(h%2), d as
            # (dhalf -> free slot, 32 -> partition)
            q8_sb = [pp.tile([128, 2, T], FP8, tag=f"q8{p}", name=f"q8{p}") for p in range(NP)]
            k8_sb = [pp.tile([128, 2, T], FP8, tag=f"k8{p}", name=f"k8{p}") for p in range(NP)]
            oT_sb = [[pp.tile([128, 512], BF16, tag=f"o{p}_{j}", name=f"o{p}_{j}") for j in range(NJ)]
                     for p in range(NP)]

            def x8_dram(c2, col0, col1):
                return xT8[256 * c2:256 * (c2 + 1), col0:col1].rearrange(
                    "(i p) t -> p i t", i=2)

            def w8_dram(wsrc, c2):
                return wsrc[256 * c2:256 * (c2 + 1), :].rearrange(
                    "(i p) d -> p i d", i=2)

            # ---- DMA in ----
            for c in range(NE):
                nc.sync.dma_start(wv_sb[c][:], wv[c * 128:(c + 1) * 128, :])
                nc.sync.dma_start(wq_sb[c][:], wq[c * 128:(c + 1) * 128, :])
                nc.sync.dma_start(wk_sb[c][:], wk[c * 128:(c + 1) * 128, :])
                nc.sync.dma_start(xT_sb[c][:, 0:512], xT[c * 128:(c + 1) * 128, 0:512])
            nc.sync.dma_start(mask_sb[:], maskb[:])
            for c2 in range(NE2):
                nc.sync.dma_start(w8q_sb[c2][:], w8_dram(w8q, c2))
                nc.sync.dma_start(w8k_sb[c2][:], w8_dram(w8k, c2))
            for w in range(1, 4):
                for c2 in range(NE2):
                    nc.sync.dma_start(x8_sb[c2][:, :, (w - 1) * 512:w * 512],
                                      x8_dram(c2, w * 512, (w + 1) * 512))
                for c in range(NE):
                    nc.sync.dma_start(xT_sb[c][:, w * 512:(w + 1) * 512],
                                      xT[c * 128:(c + 1) * 128, w * 512:(w + 1) * 512])
            for p in range(NP):
                nc.sync.dma_start(wpT_sb[p][:], wpT[p * 128:(p + 1) * 128, :])

            # ---- emission helpers ----
            def fold_dmas(dst, p, jcol, tmp):
                # tmp [128,512] fp8 rows (h, dhalf, 32) -> dst[p] [64h:64h+32, half, cols]
                # plain-slice sources only: a partition-rearranged SBUF source
                # AP breaks subtile dependency tracking (DMA races the write)
                for hh in range(2):
                    for half in range(2):
                        nc.sync.dma_start(
                            dst[p][64 * hh:64 * hh + 32, half, jcol:jcol + 512],
                            tmp[hh * 64 + half * 32:hh * 64 + half * 32 + 32, :])

            def j0_proj(p):
                for wsb, bdst, is_k in ((wq_sb, qb_sb, False), (wk_sb, kb_sb, True)):
                    ps = ps_mm.tile([128, 512], F32, tag="mm", name="mmq")
                    for c in range(NE):
                        nc.tensor.matmul(ps[:], wsb[c][:, p * 128:(p + 1) * 128],
                                         xT_sb[c][:, 0:512],
                                         start=(c == 0), stop=(c == NE - 1))
                    nc.vector.tensor_copy(bdst[p][:], ps[:])
                    if is_k:  # also fold into k8 (x32 to match fp8-side scale)
                        tmp = qtmp.tile([128, 512], FP8, tag="qt", name="qt")
                        nc.vector.tensor_scalar_mul(tmp[:], ps[:], WSCALE)
                        fold_dmas(k8_sb, p, 0, tmp)

            def jn_proj(j, p, w8_sb, dst):
                ps = ps_mm.tile([128, 512], F32, tag="mm", name="mmq")
                for c2 in range(NE2):
                    nc.tensor.matmul(
                        ps[:], w8_sb[c2][:, :, p * 128:(p + 1) * 128],
                        x8_sb[c2][:, :, (j - 1) * 512:j * 512],
                        start=(c2 == 0), stop=(c2 == NE2 - 1), perf_mode=DR)
                tmp = qtmp.tile([128, 512], FP8, tag="qt", name="qt")
                nc.vector.tensor_copy(tmp[:], ps[:])
                fold_dmas(dst, p, j * 512, tmp)

            def v_proj(i):
                ps = ps_mm.tile([128, HL * D], F32, tag="mm", name="mmv")
                for c in range(NE):
                    nc.tensor.matmul(ps[:], xT_sb[c][:, i * 128:(i + 1) * 128],
                                     wv_sb[c][:], start=(c == 0), stop=(c == NE - 1))
                v3 = v_sb[i][:].rearrange("p (h d) -> p h d", d=65)
                nc.gpsimd.memset(v_sb[i][:], 1.0)
                nc.vector.tensor_copy(
                    v3[:, :, 0:64], ps[:].rearrange("p (h d) -> p h d", d=64))
                if i < 12:
                    cp, ci = i // 2, i % 2
                    if ci == 0:
                        nc.gpsimd.memset(v8_sb[cp][:], 1.0)
                    nc.vector.tensor_copy(
                        v8_sb[cp][:, :, ci, 0:64],
                        ps[:].rearrange("p (h d) -> p h d", d=64))

            def attn(j, h):
                p, hh = h // 2, h % 2
                rb = 64 * hh
                oT_ps = ps_oT.tile([65, 512], F32, tag="oT", name="oTps")
                nblk = 4 * (j + 1)
                npair = nblk // 2
                for cp in range(npair):
                    offdiag = j > 0 and cp < 2 * j
                    cw = []
                    for ci in range(2):
                        c = 2 * cp + ci
                        r = max(0, (c - 4 * j) * 128)
                        cw.append((c, r, 512 - r))
                    sT = ps_sT.tile([128, 2, 512], F32, tag="sT", name="sTps")
                    for ci, (c, r, w) in enumerate(cw):
                        if j == 0:
                            nc.tensor.matmul(
                                sT[:, ci, 0:w],
                                kb_sb[p][rb:rb + 64, c * 128:(c + 1) * 128],
                                qb_sb[p][rb:rb + 64, r:512],
                                start=True, stop=True)
                        else:
                            nc.tensor.matmul(
                                sT[:, ci, 0:w],
                                k8_sb[p][rb:rb + 32, :, c * 128:(c + 1) * 128],
                                q8_sb[p][rb:rb + 32, :, j * 512 + r:(j + 1) * 512],
                                start=True, stop=True, perf_mode=DR)
                    scale = EXPSCALE if j == 0 else EXPSCALE8
                    if offdiag:
                        et8 = expp8.tile([128, 2, 512], FP8, tag="e8", name="e8")
                        nc.scalar.activation(et8[:], sT[:], Exp, scale=scale)
                        nc.tensor.matmul(
                            oT_ps[:, 0:512],
                            v8_sb[cp][:, h, :, 0:65],
                            et8[:],
                            start=(cp == 0), stop=False,
                            perf_mode=DR, skip_group_check=True)
                        continue
                    et = expp.tile([128, 2, 512], BF16, tag="et", name="et")
                    w0, w1 = cw[0][2], cw[1][2]
                    if w0 == 512:       # diag pair 1: widths 512, 384
                        sflat = sT[:].rearrange("p a b -> p (a b)")
                        eflat = et[:].rearrange("p a b -> p (a b)")
                        nc.scalar.activation(eflat[:, 0:896], sflat[:, 0:896],
                                             Exp, scale=scale)
                    else:               # diag pair 2: widths 256, 128
                        nc.scalar.activation(et[:, :, 0:128], sT[:, :, 0:128],
                                             Exp, scale=scale)
                        nc.scalar.activation(et[:, 0, 128:256], sT[:, 0, 128:256],
                                             Exp, scale=scale)
                    for ci, (c, r, w) in enumerate(cw):
                        nc.vector.tensor_mul(et[:, ci, 0:128],
                                             et[:, ci, 0:128], mask_sb[:])
                    v3s = [v_sb[c][:].rearrange("p (h d) -> p h d", d=65)
                           for c, _, _ in cw]
                    for ci, (c, r, w) in enumerate(cw):
                        nc.tensor.matmul(oT_ps[:, r:512],
                                         v3s[ci][:, h, :],
                                         et[:, ci, 0:w],
                                         start=(c == 0), stop=(c == nblk - 1),
                                         skip_group_check=True)
                zi = zpool.tile([1, 512], F32, tag="zi", name="zi")
                nc.vector.reciprocal(zi[:], oT_ps[64:65, :])
                zb = zpool.tile([64, 512], F32, tag="zb", name="zb")
                nc.gpsimd.partition_broadcast(zb[:], zi[:])
                nc.vector.tensor_mul(oT_sb[p][j][64 * hh:64 * hh + 64, :],
                                     oT_ps[0:64, :], zb[:])

            def out_proj(j, t):
                ob = outp.tile([128, E], F32, tag="ob", name="ob")
                for n in range(2):
                    ps = ps_mm.tile([128, 512], F32, tag="mm", name="mmo")
                    for p in range(NP):
                        nc.tensor.matmul(
                            ps[:],
                            oT_sb[p][j][:, (t % 4) * 128:(t % 4 + 1) * 128],
                            wpT_sb[p][:, n * 512:(n + 1) * 512],
                            start=(p == 0), stop=(p == NP - 1))
                    nc.vector.tensor_copy(ob[:, n * 512:(n + 1) * 512], ps[:])
                nc.sync.dma_start(out[t * 128:(t + 1) * 128, :], ob[:])

            # ---- emission schedule ----
            for p in range(NP):
                j0_proj(p)
            for i in range(4):
                v_proj(i)

            # filler queues per j-loop (popped between heads to keep PE busy
            # while ACT works)
            fillers = {
                0: ([("v", 4), ("v", 5), ("v", 6), ("v", 7),
                     ("pq", 1, 0), ("pk", 1, 0), ("v", 8), ("v", 9),
                     ("pq", 1, 1), ("pk", 1, 1), ("v", 10), ("v", 11),
                     ("pq", 1, 2), ("pk", 1, 2), ("v", 12), ("v", 13),
                     ("pq", 1, 3), ("pk", 1, 3), ("v", 14), ("v", 15)]),
                1: ([("o", 0, 0), ("pq", 2, 0), ("pk", 2, 0), ("o", 0, 1),
                     ("pq", 2, 1), ("pk", 2, 1), ("o", 0, 2),
                     ("pq", 2, 2), ("pk", 2, 2), ("o", 0, 3),
                     ("pq", 2, 3), ("pk", 2, 3)]),
                2: ([("o", 1, 0), ("pq", 3, 0), ("pk", 3, 0), ("o", 1, 1),
                     ("pq", 3, 1), ("pk", 3, 1), ("o", 1, 2),
                     ("pq", 3, 2), ("pk", 3, 2), ("o", 1, 3),
                     ("pq", 3, 3), ("pk", 3, 3)]),
                3: ([("o", 2, 0), ("o", 2, 1), ("o", 2, 2), ("o", 2, 3)]),
            }

            def run_filler(f):
                if f[0] == "v":
                    v_proj(f[1])
                elif f[0] == "pq":
                    jn_proj(f[1], f[2], w8q_sb, q8_sb)
                elif f[0] == "pk":
                    jn_proj(f[1], f[2], w8k_sb, k8_sb)
                elif f[0] == "o":
                    out_proj(f[1], f[1] * 4 + f[2])

            for j in range(NJ):
                q = list(fillers[j])
                per = (len(q) + HL - 1) // HL
                for h in range(HL):
                    attn(j, h)
                    for _ in range(per):
                        if q:
                            run_filler(q.pop(0))
                while q:
                    run_filler(q.pop(0))
            for t in range(12, 16):
                out_proj(3, t)

    nc.compile()
    return nc


def _mask_np():
    f = np.arange(128)[None, :]
    p = np.arange(128)[:, None]
    return (f >= p).astype(bfnp)


def kernel(x, Wq, Wk, Wv, Wp, bp):
    x = np.asarray(x, dtype=np.float32)
    Wq = np.asarray(Wq, dtype=np.float32)
    Wk = np.asarray(Wk, dtype=np.float32)
    Wv = np.asarray(Wv, dtype=np.float32)
    Wp = np.asarray(Wp, dtype=np.float32)
    bp = np.asarray(bp, dtype=np.float32)

    if "nc" not in _CACHE:
        _CACHE["nc"] = _build()
    nc = _CACHE["nc"]

    mask = _mask_np()
    WpT = np.ascontiguousarray(Wp.T).astype(bfnp)  # [E(hd), E(n)]
    xTs = [np.ascontiguousarray(x[b].T) for b in range(B)]

    def wslice(W, h0, scale, dt):  # [H,E,D] -> [E, 8*64] col = local head*64+d
        return np.ascontiguousarray(
            (W[h0:h0 + HL] * scale).transpose(1, 0, 2).reshape(E, HL * D)).astype(dt)

    in_maps = []
    for c in range(8):
        b, hhf = c // 2, c % 2
        h0 = hhf * HL
        in_maps.append({
            "xT": xTs[b].astype(bfnp),
            "xT8": xTs[b].astype(fp8np),
            "wq": wslice(Wq, h0, 1.0, bfnp),
            "wk": wslice(Wk, h0, 1.0, bfnp),
            "w8q": wslice(Wq, h0, WSCALE, fp8np),
            "w8k": wslice(Wk, h0, WSCALE, fp8np),
            "wv": wslice(Wv, h0, 1.0, bfnp),
            "wpT": np.ascontiguousarray(WpT[h0 * D:(h0 + HL) * D, :]),
            "maskb": mask,
        })

    res = run_bass_kernel_spmd(nc, in_maps, list(range(8)))
    parts = [np.asarray(res.results[c]["out"], dtype=np.float32) for c in range(8)]
    out = np.stack([parts[2 * b] + parts[2 * b + 1] for b in range(B)], axis=0)
    return (out + bp[None, None, :]).astype(np.float32)


# revision 17
# speedup vs baseline: 1.2145x; 1.0590x over previous
"""Multi-head causal attention (B=4,T=2048,E=1024,H=16,D=64) on 8 TRN2 cores.

Sharding: core c -> batch b=c//2, heads h0=(c%2)*8 (tensor-parallel over head
halves within each batch). Each core computes its 8 heads' attention and a
partial output projection (row-split Wp); host sums the two partials per
batch (+bias).

v3, selective precision:
  - tq tile j=0 (queries 0..511, the largest-|out| rows): fully bf16
    (projection, scores, PV) - zero fp8 noise on early tokens.
  - j>=1: Q/K projections and scores run fp8e4 DoubleRow (2x contraction per
    instr at 0.5 cyc/row); off-diagonal PV runs fp8 DoubleRow over key-chunk
    pairs; diagonal chunk pairs stay bf16 (mask multiply needs exactness).
    Softmax averaging over >=512 keys washes out fp8 quantization noise.
  - exp on ACT, one instruction per chunk-pair over a 2-bank PSUM tile.
  - Q/K fp8 weights pre-scaled x32 (exp scale absorbs /1024); j0 K is
    quantized to fp8 from the bf16 projection, scaled x32 on copy.
Emission interleaves V-proj / next-j projections / prev-j out-proj into the
per-head attention loop so PE has fill work while ACT (the exp bottleneck)
runs.
"""
import sys
import numpy as np

sys.path.insert(0, "/opt/trn_rl_repo")

import ml_dtypes
import concourse.bass as bass
import concourse.bacc as bacc
import concourse.mybir as mybir
from concourse import tile
from concourse.bass_utils import run_bass_kernel_spmd

B, T, E, H, D = 4, 2048, 1024, 16, 64
HL = H // 2      # 8 local heads per core
NP = HL // 2     # 4 head pairs
NJ = T // 512    # 4 tq tiles
NCK = T // 128   # 16 t chunks
NE = E // 128    # 8 e chunks (bf16)
NE2 = E // 256   # 4 e pair-chunks (fp8 DoubleRow)
BF16 = mybir.dt.bfloat16
F32 = mybir.dt.float32
FP8 = mybir.dt.float8e4
bfnp = ml_dtypes.bfloat16
fp8np = ml_dtypes.float8_e4m3
DR = mybir.MatmulPerfMode.DoubleRow
MUL = mybir.AluOpType.mult

WSCALE = 32.0
EXPSCALE8 = 0.125 / (WSCALE * WSCALE)
EXPSCALE = 0.125

_CACHE = {}


def _build():
    nc = bacc.Bacc("TRN2", target_bir_lowering=False)
    xT = nc.declare_dram_parameter("xT", [E, T], BF16, isOutput=False)
    xT8 = nc.declare_dram_parameter("xT8", [E, T], FP8, isOutput=False)
    wq = nc.declare_dram_parameter("wq", [E, HL * D], BF16, isOutput=False)
    wk = nc.declare_dram_parameter("wk", [E, HL * D], BF16, isOutput=False)
    w8q = nc.declare_dram_parameter("w8q", [E, HL * D], FP8, isOutput=False)
    w8k = nc.declare_dram_parameter("w8k", [E, HL * D], FP8, isOutput=False)
    wv = nc.declare_dram_parameter("wv", [E, HL * D], BF16, isOutput=False)
    wpT = nc.declare_dram_parameter("wpT", [HL * D, E], BF16, isOutput=False)
    maskb = nc.declare_dram_parameter("maskb", [128, 128], BF16, isOutput=False)
    out = nc.declare_dram_parameter("out", [T, E], F32, isOutput=True)

    Exp = mybir.ActivationFunctionType.Exp

    with tile.TileContext(nc) as tc:
        with (
            tc.tile_pool(name="persist", bufs=1) as pp,
            tc.tile_pool(name="qtmp", bufs=3) as qtmp,
            tc.tile_pool(name="expp", bufs=4) as expp,
            tc.tile_pool(name="expp8", bufs=4) as expp8,
            tc.tile_pool(name="zpool", bufs=4) as zpool,
            tc.tile_pool(name="outp", bufs=2) as outp,
            tc.tile_pool(name="mm", bufs=2, space=bass.MemorySpace.PSUM) as ps_mm,
            tc.tile_pool(name="sT", bufs=2, space=bass.MemorySpace.PSUM) as ps_sT,
            tc.tile_pool(name="oT", bufs=2, space=bass.MemorySpace.PSUM) as ps_oT,
        ):
            # consolidated tiles: e-chunk as a free dim so each bulk tensor
            # loads with ONE dma_start (HWDGE descriptor gen is 625ns each)
            xT_all = pp.tile([128, NE, T], BF16, tag="xTa", name="xTa")
            x8_all = pp.tile([128, NE2, 2, T - 512], FP8, tag="x8a", name="x8a")
            wq_all = pp.tile([128, NE, HL * D], BF16, tag="bqa", name="bqa")
            wk_all = pp.tile([128, NE, HL * D], BF16, tag="bka", name="bka")
            w8q_all = pp.tile([128, NE2, 2, 512], FP8, tag="wqa", name="wqa")
            w8k_all = pp.tile([128, NE2, 2, 512], FP8, tag="wka", name="wka")
            wv_all = pp.tile([128, NE, HL * D], BF16, tag="wva", name="wva")
            wpT_all = pp.tile([128, NP, E], BF16, tag="wpa", name="wpa")
            xT_sb = [xT_all[:, c] for c in range(NE)]
            x8_sb = [x8_all[:, c] for c in range(NE2)]
            wq_sb = [wq_all[:, c] for c in range(NE)]
            wk_sb = [wk_all[:, c] for c in range(NE)]
            w8q_sb = [w8q_all[:, c] for c in range(NE2)]
            w8k_sb = [w8k_all[:, c] for c in range(NE2)]
            wv_sb = [wv_all[:, c] for c in range(NE)]
            wpT_sb = [wpT_all[:, p] for p in range(NP)]
            mask_sb = pp.tile([128, 128], BF16, tag="mkb", name="mkb")
            v_sb = [pp.tile([128, HL * 65], BF16, tag=f"v{i}", name=f"v{i}") for i in range(NCK)]
            # v8: [128, HL, 2, 128] fp8 - per-head [2,128] block so the
            # DoubleRow Ldweights pair stride is 128 (s3_lw_dual_fp8 wants
            # an aligned pair stride); v at cols 0:64, ones at col 64.
            v8_sb = [pp.tile([128, HL, 2, 128], FP8, tag=f"v8{i}", name=f"v8{i}")
                     for i in range(6)]
            # j0 bf16 q/k: [128, 512]; rows = head (h%2)*64 + d
            qb_sb = [pp.tile([128, 512], BF16, tag=f"qb{p}", name=f"qb{p}") for p in range(NP)]
            kb_sb = [pp.tile([128, 512], BF16, tag=f"kb{p}", name=f"kb{p}") for p in range(NP)]
            # fp8 q/k: [128, 2, T]; head h%2 at partition base 64*(h%2), d as
            # (dhalf -> free slot, 32 -> partition)
            q8_sb = [pp.tile([128, 2, T], FP8, tag=f"q8{p}", name=f"q8{p}") for p in range(NP)]
            k8_sb = [pp.tile([128, 2, T], FP8, tag=f"k8{p}", name=f"k8{p}") for p in range(NP)]
            oT_sb = [[pp.tile([128, 512], BF16, tag=f"o{p}_{j}", name=f"o{p}_{j}") for j in range(NJ)]
                     for p in range(NP)]

            # ---- DMA in (one dma_start per bulk tensor / wave) ----
            def chunked(dram, c):  # [E, X] -> [128, c, X]
                return dram[:].rearrange("(c p) t -> p c t", c=c)

            def paired(dram, col0, col1):  # [E, X] -> [128, NE2, 2, cols]
                return dram[:, col0:col1].rearrange(
                    "(c i p) t -> p c i t", c=NE2, i=2)

            nc.sync.dma_start(xT_all[:, :, 0:512],
                              xT[:, 0:512].rearrange("(c p) t -> p c t", c=NE))
            nc.sync.dma_start(wq_all[:], chunked(wq, NE))
            nc.sync.dma_start(wk_all[:], chunked(wk, NE))
            nc.sync.dma_start(wv_all[:], chunked(wv, NE))
            nc.sync.dma_start(mask_sb[:], maskb[:])
            nc.sync.dma_start(w8q_all[:], paired(w8q, 0, 512))
            nc.sync.dma_start(w8k_all[:], paired(w8k, 0, 512))
            for w in range(1, 4):
                nc.sync.dma_start(x8_all[:, :, :, (w - 1) * 512:w * 512],
                                  paired(xT8, w * 512, (w + 1) * 512))
                nc.sync.dma_start(
                    xT_all[:, :, w * 512:(w + 1) * 512],
                    xT[:, w * 512:(w + 1) * 512].rearrange("(c p) t -> p c t", c=NE))
            nc.sync.dma_start(wpT_all[:], chunked(wpT, NP))

            # ---- emission helpers ----
            def fold_dmas(dst, p, jcol, tmp):
                # tmp [128,512] fp8 rows (h, dhalf, 32) -> dst[p] [64h:64h+32, half, cols]
                # plain-slice sources only: a partition-rearranged SBUF source
                # AP breaks subtile dependency tracking (DMA races the write)
                for hh in range(2):
                    for half in range(2):
                        nc.sync.dma_start(
                            dst[p][64 * hh:64 * hh + 32, half, jcol:jcol + 512],
                            tmp[hh * 64 + half * 32:hh * 64 + half * 32 + 32, :])

            def j0_proj(p):
                for wsb, bdst, is_k in ((wq_sb, qb_sb, False), (wk_sb, kb_sb, True)):
                    ps = ps_mm.tile([128, 512], F32, tag="mm", name="mmq")
                    for c in range(NE):
                        nc.tensor.matmul(ps[:], wsb[c][:, p * 128:(p + 1) * 128],
                                         xT_sb[c][:, 0:512],
                                         start=(c == 0), stop=(c == NE - 1))
                    nc.vector.tensor_copy(bdst[p][:], ps[:])
                    if is_k:  # also fold into k8 (x32 to match fp8-side scale)
                        tmp = qtmp.tile([128, 512], FP8, tag="qt", name="qt")
                        nc.vector.tensor_scalar_mul(tmp[:], ps[:], WSCALE)
                        fold_dmas(k8_sb, p, 0, tmp)

            def jn_proj(j, p, w8_sb, dst):
                ps = ps_mm.tile([128, 512], F32, tag="mm", name="mmq")
                for c2 in range(NE2):
                    nc.tensor.matmul(
                        ps[:], w8_sb[c2][:, :, p * 128:(p + 1) * 128],
                        x8_sb[c2][:, :, (j - 1) * 512:j * 512],
                        start=(c2 == 0), stop=(c2 == NE2 - 1), perf_mode=DR)
                tmp = qtmp.tile([128, 512], FP8, tag="qt", name="qt")
                nc.vector.tensor_copy(tmp[:], ps[:])
                fold_dmas(dst, p, j * 512, tmp)

            def v_proj(i):
                ps = ps_mm.tile([128, HL * D], F32, tag="mm", name="mmv")
                for c in range(NE):
                    nc.tensor.matmul(ps[:], xT_sb[c][:, i * 128:(i + 1) * 128],
                                     wv_sb[c][:], start=(c == 0), stop=(c == NE - 1))
                v3 = v_sb[i][:].rearrange("p (h d) -> p h d", d=65)
                nc.gpsimd.memset(v_sb[i][:], 1.0)
                nc.vector.tensor_copy(
                    v3[:, :, 0:64], ps[:].rearrange("p (h d) -> p h d", d=64))
                if i < 12:
                    cp, ci = i // 2, i % 2
                    if ci == 0:
                        nc.gpsimd.memset(v8_sb[cp][:], 1.0)
                    nc.vector.tensor_copy(
                        v8_sb[cp][:, :, ci, 0:64],
                        ps[:].rearrange("p (h d) -> p h d", d=64))

            def attn(j, h):
                p, hh = h // 2, h % 2
                rb = 64 * hh
                oT_ps = ps_oT.tile([65, 512], F32, tag="oT", name="oTps")
                nblk = 4 * (j + 1)
                npair = nblk // 2
                for cp in range(npair):
                    offdiag = j > 0 and cp < 2 * j
                    cw = []
                    for ci in range(2):
                        c = 2 * cp + ci
                        r = max(0, (c - 4 * j) * 128)
                        cw.append((c, r, 512 - r))
                    sT = ps_sT.tile([128, 2, 512], F32, tag="sT", name="sTps")
                    for ci, (c, r, w) in enumerate(cw):
                        if j == 0:
                            nc.tensor.matmul(
                                sT[:, ci, 0:w],
                                kb_sb[p][rb:rb + 64, c * 128:(c + 1) * 128],
                                qb_sb[p][rb:rb + 64, r:512],
                                start=True, stop=True)
                        else:
                            nc.tensor.matmul(
                                sT[:, ci, 0:w],
                                k8_sb[p][rb:rb + 32, :, c * 128:(c + 1) * 128],
                                q8_sb[p][rb:rb + 32, :, j * 512 + r:(j + 1) * 512],
                                start=True, stop=True, perf_mode=DR)
                    scale = EXPSCALE if j == 0 else EXPSCALE8
                    if offdiag:
                        et8 = expp8.tile([128, 2, 512], FP8, tag="e8", name="e8")
                        nc.scalar.activation(et8[:], sT[:], Exp, scale=scale)
                        nc.tensor.matmul(
                            oT_ps[:, 0:512],
                            v8_sb[cp][:, h, :, 0:65],
                            et8[:],
                            start=(cp == 0), stop=False,
                            perf_mode=DR, skip_group_check=True)
                        continue
                    et = expp.tile([128, 2, 512], BF16, tag="et", name="et")
                    w0, w1 = cw[0][2], cw[1][2]
                    if w0 == 512:       # diag pair 1: widths 512, 384
                        sflat = sT[:].rearrange("p a b -> p (a b)")
                        eflat = et[:].rearrange("p a b -> p (a b)")
                        nc.scalar.activation(eflat[:, 0:896], sflat[:, 0:896],
                                             Exp, scale=scale)
                    else:               # diag pair 2: widths 256, 128
                        nc.scalar.activation(et[:, :, 0:128], sT[:, :, 0:128],
                                             Exp, scale=scale)
                        nc.scalar.activation(et[:, 0, 128:256], sT[:, 0, 128:256],
                                             Exp, scale=scale)
                    for ci, (c, r, w) in enumerate(cw):
                        nc.vector.tensor_mul(et[:, ci, 0:128],
                                             et[:, ci, 0:128], mask_sb[:])
                    v3s = [v_sb[c][:].rearrange("p (h d) -> p h d", d=65)
                           for c, _, _ in cw]
                    for ci, (c, r, w) in enumerate(cw):
                        nc.tensor.matmul(oT_ps[:, r:512],
                                         v3s[ci][:, h, :],
                                         et[:, ci, 0:w],
                                         start=(c == 0), stop=(c == nblk - 1),
                                         skip_group_check=True)
                zi = zpool.tile([1, 512], F32, tag="zi", name="zi")
                nc.vector.reciprocal(zi[:], oT_ps[64:65, :])
                zb = zpool.tile([64, 512], F32, tag="zb", name="zb")
                nc.gpsimd.partition_broadcast(zb[:], zi[:])
                nc.vector.tensor_mul(oT_sb[p][j][64 * hh:64 * hh + 64, :],
                                     oT_ps[0:64, :], zb[:])

            def out_proj(j, t):
                ob = outp.tile([128, E], F32, tag="ob", name="ob")
                for n in range(2):
                    ps = ps_mm.tile([128, 512], F32, tag="mm", name="mmo")
                    for p in range(NP):
                        nc.tensor.matmul(
                            ps[:],
                            oT_sb[p][j][:, (t % 4) * 128:(t % 4 + 1) * 128],
                            wpT_sb[p][:, n * 512:(n + 1) * 512],
                            start=(p == 0), stop=(p == NP - 1))
                    nc.vector.tensor_copy(ob[:, n * 512:(n + 1) * 512], ps[:])
                nc.sync.dma_start(out[t * 128:(t + 1) * 128, :], ob[:])

            # ---- emission schedule ----
            for p in range(NP):
                j0_proj(p)
            for i in range(4):
                v_proj(i)

            # filler queues per j-loop (popped between heads to keep PE busy
            # while ACT works)
            fillers = {
                0: ([("pq", 1, 0), ("pk", 1, 0), ("v", 4), ("v", 5),
                     ("pq", 1, 1), ("pk", 1, 1), ("v", 6), ("v", 7),
                     ("pq", 1, 2), ("pk", 1, 2), ("v", 8), ("v", 9),
                     ("pq", 1, 3), ("pk", 1, 3), ("v", 10), ("v", 11),
                     ("v", 12), ("v", 13), ("v", 14), ("v", 15)]),
                1: ([("pq", 2, 0), ("pk", 2, 0), ("o", 0, 0),
                     ("pq", 2, 1), ("pk", 2, 1), ("o", 0, 1),
                     ("pq", 2, 2), ("pk", 2, 2), ("o", 0, 2),
                     ("pq", 2, 3), ("pk", 2, 3), ("o", 0, 3)]),
                2: ([("pq", 3, 0), ("pk", 3, 0), ("o", 1, 0),
                     ("pq", 3, 1), ("pk", 3, 1), ("o", 1, 1),
                     ("pq", 3, 2), ("pk", 3, 2), ("o", 1, 2),
                     ("pq", 3, 3), ("pk", 3, 3), ("o", 1, 3)]),
                3: ([("o", 2, 0), ("o", 2, 1), ("o", 2, 2), ("o", 2, 3)]),
            }

            def run_filler(f):
                if f[0] == "v":
                    v_proj(f[1])
                elif f[0] == "pq":
                    jn_proj(f[1], f[2], w8q_sb, q8_sb)
                elif f[0] == "pk":
                    jn_proj(f[1], f[2], w8k_sb, k8_sb)
                elif f[0] == "o":
                    out_proj(f[1], f[1] * 4 + f[2])

            for j in range(NJ):
                q = list(fillers[j])
                per = (len(q) + HL - 1) // HL
                for h in range(HL):
                    attn(j, h)
                    for _ in range(per):
                        if q:
                            run_filler(q.pop(0))
                while q:
                    run_filler(q.pop(0))
            for t in range(12, 16):
                out_proj(3, t)

    nc.compile()
    return nc


def _mask_np():
    f = np.arange(128)[None, :]
    p = np.arange(128)[:, None]
    return (f >= p).astype(bfnp)


def kernel(x, Wq, Wk, Wv, Wp, bp):
    x = np.asarray(x, dtype=np.float32)
    Wq = np.asarray(Wq, dtype=np.float32)
    Wk = np.asarray(Wk, dtype=np.float32)
    Wv = np.asarray(Wv, dtype=np.float32)
    Wp = np.asarray(Wp, dtype=np.float32)
    bp = np.asarray(bp, dtype=np.float32)

    if "nc" not in _CACHE:
        _CACHE["nc"] = _build()
    nc = _CACHE["nc"]

    mask = _mask_np()
    WpT = np.ascontiguousarray(Wp.T).astype(bfnp)  # [E(hd), E(n)]
    xTs = [np.ascontiguousarray(x[b].T) for b in range(B)]

    def wslice(W, h0, scale, dt):  # [H,E,D] -> [E, 8*64] col = local head*64+d
        return np.ascontiguousarray(
            (W[h0:h0 + HL] * scale).transpose(1, 0, 2).reshape(E, HL * D)).astype(dt)

    in_maps = []
    for c in range(8):
        b, hhf = c // 2, c % 2
        h0 = hhf * HL
        in_maps.append({
            "xT": xTs[b].astype(bfnp),
            "xT8": xTs[b].astype(fp8np),
            "wq": wslice(Wq, h0, 1.0, bfnp),
            "wk": wslice(Wk, h0, 1.0, bfnp),
            "w8q": wslice(Wq, h0, WSCALE, fp8np),
            "w8k": wslice(Wk, h0, WSCALE, fp8np),
            "wv": wslice(Wv, h0, 1.0, bfnp),
            "wpT": np.ascontiguousarray(WpT[h0 * D:(h0 + HL) * D, :]),
            "maskb": mask,
        })

    res = run_bass_kernel_spmd(nc, in_maps, list(range(8)))
    parts = [np.asarray(res.results[c]["out"], dtype=np.float32) for c in range(8)]
    out = np.stack([parts[2 * b] + parts[2 * b + 1] for b in range(B)], axis=0)
    return (out + bp[None, None, :]).astype(np.float32)


# revision 20
# speedup vs baseline: 1.2616x; 1.0388x over previous
"""Multi-head causal attention (B=4,T=2048,E=1024,H=16,D=64) on 8 TRN2 cores.

Sharding: core c -> batch b=c//2, heads h0=(c%2)*8 (tensor-parallel over head
halves within each batch). Each core computes its 8 heads' attention and a
partial output projection (row-split Wp); host sums the two partials per
batch (+bias).

v3, selective precision:
  - tq tile j=0 (queries 0..511, the largest-|out| rows): fully bf16
    (projection, scores, PV) - zero fp8 noise on early tokens.
  - j>=1: Q/K projections and scores run fp8e4 DoubleRow (2x contraction per
    instr at 0.5 cyc/row); off-diagonal PV runs fp8 DoubleRow over key-chunk
    pairs; diagonal chunk pairs stay bf16 (mask multiply needs exactness).
    Softmax averaging over >=512 keys washes out fp8 quantization noise.
  - exp on ACT, one instruction per chunk-pair over a 2-bank PSUM tile.
  - Q/K fp8 weights pre-scaled x32 (exp scale absorbs /1024); j0 K is
    quantized to fp8 from the bf16 projection, scaled x32 on copy.
Emission interleaves V-proj / next-j projections / prev-j out-proj into the
per-head attention loop so PE has fill work while ACT (the exp bottleneck)
runs.
"""
import sys
import numpy as np

sys.path.insert(0, "/opt/trn_rl_repo")

import ml_dtypes
import concourse.bass as bass
import concourse.bacc as bacc
import concourse.mybir as mybir
from concourse import tile
from concourse.bass_utils import run_bass_kernel_spmd

B, T, E, H, D = 4, 2048, 1024, 16, 64
HL = H // 2      # 8 local heads per core
NP = HL // 2     # 4 head pairs
NJ = T // 512    # 4 tq tiles
NCK = T // 128   # 16 t chunks
NE = E // 128    # 8 e chunks (bf16)
NE2 = E // 256   # 4 e pair-chunks (fp8 DoubleRow)
BF16 = mybir.dt.bfloat16
F32 = mybir.dt.float32
FP8 = mybir.dt.float8e4
bfnp = ml_dtypes.bfloat16
fp8np = ml_dtypes.float8_e4m3
DR = mybir.MatmulPerfMode.DoubleRow
MUL = mybir.AluOpType.mult

WSCALE = 32.0
EXPSCALE8 = 0.125 / (WSCALE * WSCALE)
EXPSCALE = 0.125

_CACHE = {}


def _build():
    nc = bacc.Bacc("TRN2", target_bir_lowering=False)
    xT = nc.declare_dram_parameter("xT", [E, T], BF16, isOutput=False)
    xT8 = nc.declare_dram_parameter("xT8", [E, T], FP8, isOutput=False)
    wq = nc.declare_dram_parameter("wq", [E, HL * D], BF16, isOutput=False)
    wk = nc.declare_dram_parameter("wk", [E, HL * D], BF16, isOutput=False)
    w8q = nc.declare_dram_parameter("w8q", [E, HL * D], FP8, isOutput=False)
    w8k = nc.declare_dram_parameter("w8k", [E, HL * D], FP8, isOutput=False)
    wv = nc.declare_dram_parameter("wv", [E, HL * D], BF16, isOutput=False)
    wpT = nc.declare_dram_parameter("wpT", [HL * D, E], BF16, isOutput=False)
    maskb = nc.declare_dram_parameter("maskb", [128, 128], BF16, isOutput=False)
    out = nc.declare_dram_parameter("out", [T, E], F32, isOutput=True)

    Exp = mybir.ActivationFunctionType.Exp

    with tile.TileContext(nc) as tc:
        with (
            tc.tile_pool(name="persist", bufs=1) as pp,
            tc.tile_pool(name="qtmp", bufs=3) as qtmp,
            tc.tile_pool(name="expp", bufs=4) as expp,
            tc.tile_pool(name="expp8", bufs=4) as expp8,
            tc.tile_pool(name="zpool", bufs=4) as zpool,
            tc.tile_pool(name="outp", bufs=2) as outp,
            tc.tile_pool(name="mm", bufs=2, space=bass.MemorySpace.PSUM) as ps_mm,
            tc.tile_pool(name="sT", bufs=2, space=bass.MemorySpace.PSUM) as ps_sT,
            tc.tile_pool(name="oT", bufs=2, space=bass.MemorySpace.PSUM) as ps_oT,
        ):
            # consolidated tiles: e-chunk as a free dim so each bulk tensor
            # loads with ONE dma_start (HWDGE descriptor gen is 625ns each)
            xT_all = pp.tile([128, NE, T], BF16, tag="xTa", name="xTa")
            x8_all = pp.tile([128, NE2, 2, T - 512], FP8, tag="x8a", name="x8a")
            wq_all = pp.tile([128, NE, HL * D], BF16, tag="bqa", name="bqa")
            wk_all = pp.tile([128, NE, HL * D], BF16, tag="bka", name="bka")
            w8q_all = pp.tile([128, NE2, 2, 512], FP8, tag="wqa", name="wqa")
            w8k_all = pp.tile([128, NE2, 2, 512], FP8, tag="wka", name="wka")
            wv_all = pp.tile([128, NE, HL * D], BF16, tag="wva", name="wva")
            wpT_all = pp.tile([128, NP, E], BF16, tag="wpa", name="wpa")
            xT_sb = [xT_all[:, c] for c in range(NE)]
            x8_sb = [x8_all[:, c] for c in range(NE2)]
            wq_sb = [wq_all[:, c] for c in range(NE)]
            wk_sb = [wk_all[:, c] for c in range(NE)]
            w8q_sb = [w8q_all[:, c] for c in range(NE2)]
            w8k_sb = [w8k_all[:, c] for c in range(NE2)]
            wv_sb = [wv_all[:, c] for c in range(NE)]
            wpT_sb = [wpT_all[:, p] for p in range(NP)]
            mask_sb = pp.tile([128, 128], BF16, tag="mkb", name="mkb")
            v_sb = [pp.tile([128, HL * 65], BF16, tag=f"v{i}", name=f"v{i}") for i in range(NCK)]
            # v8: [128, HL, 2, 128] fp8 - per-head [2,128] block so the
            # DoubleRow Ldweights pair stride is 128 (s3_lw_dual_fp8 wants
            # an aligned pair stride); v at cols 0:64, ones at col 64.
            v8_sb = [pp.tile([128, HL, 2, 128], FP8, tag=f"v8{i}", name=f"v8{i}")
                     for i in range(6)]
            # j0 bf16 q/k: [128, 512]; rows = head (h%2)*64 + d
            qb_sb = [pp.tile([128, 512], BF16, tag=f"qb{p}", name=f"qb{p}") for p in range(NP)]
            kb_sb = [pp.tile([128, 512], BF16, tag=f"kb{p}", name=f"kb{p}") for p in range(NP)]
            # fp8 q/k: [128, 2, T]; head h%2 at partition base 64*(h%2), d as
            # (dhalf -> free slot, 32 -> partition)
            q8_sb = [pp.tile([128, 2, T], FP8, tag=f"q8{p}", name=f"q8{p}") for p in range(NP)]
            k8_sb = [pp.tile([128, 2, T], FP8, tag=f"k8{p}", name=f"k8{p}") for p in range(NP)]
            oT_sb = [[pp.tile([128, 512], BF16, tag=f"o{p}_{j}", name=f"o{p}_{j}") for j in range(NJ)]
                     for p in range(NP)]

            # ---- DMA in (one dma_start per bulk tensor / wave) ----
            def chunked(dram, c):  # [E, X] -> [128, c, X]
                return dram[:].rearrange("(c p) t -> p c t", c=c)

            def paired(dram, col0, col1):  # [E, X] -> [128, NE2, 2, cols]
                return dram[:, col0:col1].rearrange(
                    "(c i p) t -> p c i t", c=NE2, i=2)

            nc.sync.dma_start(wq_all[:], chunked(wq, NE))
            nc.sync.dma_start(wk_all[:], chunked(wk, NE))
            nc.sync.dma_start(xT_all[:, :, 0:512],
                              xT[:, 0:512].rearrange("(c p) t -> p c t", c=NE))
            nc.sync.dma_start(wv_all[:], chunked(wv, NE))
            nc.sync.dma_start(mask_sb[:], maskb[:])
            nc.sync.dma_start(w8q_all[:], paired(w8q, 0, 512))
            nc.sync.dma_start(w8k_all[:], paired(w8k, 0, 512))
            for w in range(1, 4):
                nc.sync.dma_start(x8_all[:, :, :, (w - 1) * 512:w * 512],
                                  paired(xT8, w * 512, (w + 1) * 512))
                nc.sync.dma_start(
                    xT_all[:, :, w * 512:(w + 1) * 512],
                    xT[:, w * 512:(w + 1) * 512].rearrange("(c p) t -> p c t", c=NE))
            nc.sync.dma_start(wpT_all[:], chunked(wpT, NP))

            # ---- emission helpers ----
            def fold_dmas(dst, p, jcol, tmp):
                # tmp [128,512] fp8 rows (h, dhalf, 32) -> dst[p] [64h:64h+32, half, cols]
                # plain-slice sources only: a partition-rearranged SBUF source
                # AP breaks subtile dependency tracking (DMA races the write)
                for hh in range(2):
                    for half in range(2):
                        nc.sync.dma_start(
                            dst[p][64 * hh:64 * hh + 32, half, jcol:jcol + 512],
                            tmp[hh * 64 + half * 32:hh * 64 + half * 32 + 32, :])

            def j0_proj(p):
                for wsb, bdst, is_k in ((wq_sb, qb_sb, False), (wk_sb, kb_sb, True)):
                    ps = ps_mm.tile([128, 512], F32, tag="mm", name="mmq")
                    for c in range(NE):
                        nc.tensor.matmul(ps[:], wsb[c][:, p * 128:(p + 1) * 128],
                                         xT_sb[c][:, 0:512],
                                         start=(c == 0), stop=(c == NE - 1))
                    nc.vector.tensor_copy(bdst[p][:], ps[:])
                    if is_k:  # also fold into k8 (x32 to match fp8-side scale)
                        tmp = qtmp.tile([128, 512], FP8, tag="qt", name="qt")
                        nc.vector.tensor_scalar_mul(tmp[:], ps[:], WSCALE)
                        fold_dmas(k8_sb, p, 0, tmp)

            def jn_proj(j, p, w8_sb, dst):
                ps = ps_mm.tile([128, 512], F32, tag="mm", name="mmq")
                for c2 in range(NE2):
                    nc.tensor.matmul(
                        ps[:], w8_sb[c2][:, :, p * 128:(p + 1) * 128],
                        x8_sb[c2][:, :, (j - 1) * 512:j * 512],
                        start=(c2 == 0), stop=(c2 == NE2 - 1), perf_mode=DR)
                tmp = qtmp.tile([128, 512], FP8, tag="qt", name="qt")
                nc.vector.tensor_copy(tmp[:], ps[:])
                fold_dmas(dst, p, j * 512, tmp)

            def v_proj(i):
                ps = ps_mm.tile([128, HL * D], F32, tag="mm", name="mmv")
                for c in range(NE):
                    nc.tensor.matmul(ps[:], xT_sb[c][:, i * 128:(i + 1) * 128],
                                     wv_sb[c][:], start=(c == 0), stop=(c == NE - 1))
                v3 = v_sb[i][:].rearrange("p (h d) -> p h d", d=65)
                nc.gpsimd.memset(v_sb[i][:], 1.0)
                nc.vector.tensor_copy(
                    v3[:, :, 0:64], ps[:].rearrange("p (h d) -> p h d", d=64))
                if i < 12:
                    cp, ci = i // 2, i % 2
                    if ci == 0:
                        nc.gpsimd.memset(v8_sb[cp][:], 1.0)
                    nc.vector.tensor_copy(
                        v8_sb[cp][:, :, ci, 0:64],
                        ps[:].rearrange("p (h d) -> p h d", d=64))

            def attn(j, h):
                p, hh = h // 2, h % 2
                rb = 64 * hh
                oT_ps = ps_oT.tile([65, 512], F32, tag="oT", name="oTps")
                nblk = 4 * (j + 1)
                npair = nblk // 2
                for cp in range(npair):
                    offdiag = j > 0 and cp < 2 * j
                    cw = []
                    for ci in range(2):
                        c = 2 * cp + ci
                        r = max(0, (c - 4 * j) * 128)
                        cw.append((c, r, 512 - r))
                    sT = ps_sT.tile([128, 2, 512], F32, tag="sT", name="sTps")
                    for ci, (c, r, w) in enumerate(cw):
                        if j == 0:
                            nc.tensor.matmul(
                                sT[:, ci, 0:w],
                                kb_sb[p][rb:rb + 64, c * 128:(c + 1) * 128],
                                qb_sb[p][rb:rb + 64, r:512],
                                start=True, stop=True)
                        else:
                            nc.tensor.matmul(
                                sT[:, ci, 0:w],
                                k8_sb[p][rb:rb + 32, :, c * 128:(c + 1) * 128],
                                q8_sb[p][rb:rb + 32, :, j * 512 + r:(j + 1) * 512],
                                start=True, stop=True, perf_mode=DR)
                    scale = EXPSCALE if j == 0 else EXPSCALE8
                    if offdiag:
                        et8 = expp8.tile([128, 2, 512], FP8, tag="e8", name="e8")
                        nc.scalar.activation(et8[:], sT[:], Exp, scale=scale)
                        nc.tensor.matmul(
                            oT_ps[:, 0:512],
                            v8_sb[cp][:, h, :, 0:65],
                            et8[:],
                            start=(cp == 0), stop=False,
                            perf_mode=DR, skip_group_check=True)
                        continue
                    et = expp.tile([128, 2, 512], BF16, tag="et", name="et")
                    w0, w1 = cw[0][2], cw[1][2]
                    if w0 == 512:       # diag pair 1: widths 512, 384
                        sflat = sT[:].rearrange("p a b -> p (a b)")
                        eflat = et[:].rearrange("p a b -> p (a b)")
                        nc.scalar.activation(eflat[:, 0:896], sflat[:, 0:896],
                                             Exp, scale=scale)
                    else:               # diag pair 2: widths 256, 128
                        nc.scalar.activation(et[:, :, 0:128], sT[:, :, 0:128],
                                             Exp, scale=scale)
                        nc.scalar.activation(et[:, 0, 128:256], sT[:, 0, 128:256],
                                             Exp, scale=scale)
                    for ci, (c, r, w) in enumerate(cw):
                        nc.vector.tensor_mul(et[:, ci, 0:128],
                                             et[:, ci, 0:128], mask_sb[:])
                    v3s = [v_sb[c][:].rearrange("p (h d) -> p h d", d=65)
                           for c, _, _ in cw]
                    for ci, (c, r, w) in enumerate(cw):
                        nc.tensor.matmul(oT_ps[:, r:512],
                                         v3s[ci][:, h, :],
                                         et[:, ci, 0:w],
                                         start=(c == 0), stop=(c == nblk - 1),
                                         skip_group_check=True)
                zi = zpool.tile([1, 512], F32, tag="zi", name="zi")
                nc.vector.reciprocal(zi[:], oT_ps[64:65, :])
                zb = zpool.tile([64, 512], F32, tag="zb", name="zb")
                nc.gpsimd.partition_broadcast(zb[:], zi[:])
                nc.vector.tensor_mul(oT_sb[p][j][64 * hh:64 * hh + 64, :],
                                     oT_ps[0:64, :], zb[:])

            def out_proj(j, t):
                ob = outp.tile([128, E], F32, tag="ob", name="ob")
                for n in range(2):
                    ps = ps_mm.tile([128, 512], F32, tag="mm", name="mmo")
                    for p in range(NP):
                        nc.tensor.matmul(
                            ps[:],
                            oT_sb[p][j][:, (t % 4) * 128:(t % 4 + 1) * 128],
                            wpT_sb[p][:, n * 512:(n + 1) * 512],
                            start=(p == 0), stop=(p == NP - 1))
                    nc.vector.tensor_copy(ob[:, n * 512:(n + 1) * 512], ps[:])
                nc.sync.dma_start(out[t * 128:(t + 1) * 128, :], ob[:])

            # ---- emission schedule ----
            # start attention as early as possible: only pair 0's bf16
            # projection and V chunks 0..3 gate attn(0,0); the rest of the
            # j0 projections and all later work rides in the filler slots.
            j0_proj(0)
            for i in range(4):
                v_proj(i)

            # filler queues per j-loop (popped between heads to keep PE busy
            # while ACT works); each phase's filler PE budget roughly matches
            # that phase's ACT time minus its attention PE time
            fillers = {
                0: ([("j0p", 1), ("pq", 1, 0), ("pk", 1, 0), ("j0p", 2),
                     ("j0p", 3), ("pq", 1, 1), ("pk", 1, 1), ("v", 4),
                     ("pq", 1, 2), ("pk", 1, 2), ("v", 5),
                     ("pq", 1, 3), ("pk", 1, 3), ("v", 6), ("v", 7)]),
                1: ([("pq", 2, 0), ("pk", 2, 0), ("v", 8), ("o", 0, 0),
                     ("pq", 2, 1), ("pk", 2, 1), ("v", 9), ("o", 0, 1),
                     ("pq", 2, 2), ("pk", 2, 2), ("v", 10), ("o", 0, 2),
                     ("pq", 2, 3), ("pk", 2, 3), ("v", 11), ("o", 0, 3)]),
                2: ([("pq", 3, 0), ("pk", 3, 0), ("v", 12), ("o", 1, 0),
                     ("pq", 3, 1), ("pk", 3, 1), ("v", 13), ("o", 1, 1),
                     ("pq", 3, 2), ("pk", 3, 2), ("v", 14), ("o", 1, 2),
                     ("pq", 3, 3), ("pk", 3, 3), ("v", 15), ("o", 1, 3)]),
                3: ([("o", 2, 0), ("o", 2, 1), ("o", 2, 2), ("o", 2, 3)]),
            }

            def run_filler(f):
                if f[0] == "v":
                    v_proj(f[1])
                elif f[0] == "j0p":
                    j0_proj(f[1])
                elif f[0] == "pq":
                    jn_proj(f[1], f[2], w8q_sb, q8_sb)
                elif f[0] == "pk":
                    jn_proj(f[1], f[2], w8k_sb, k8_sb)
                elif f[0] == "o":
                    out_proj(f[1], f[1] * 4 + f[2])

            for j in range(NJ):
                q = list(fillers[j])
                per = (len(q) + HL - 1) // HL
                for h in range(HL):
                    attn(j, h)
                    for _ in range(per):
                        if q:
                            run_filler(q.pop(0))
                while q:
                    run_filler(q.pop(0))
            for t in range(12, 16):
                out_proj(3, t)

    nc.compile()
    return nc


def _mask_np():
    f = np.arange(128)[None, :]
    p = np.arange(128)[:, None]
    return (f >= p).astype(bfnp)


def kernel(x, Wq, Wk, Wv, Wp, bp):
    x = np.asarray(x, dtype=np.float32)
    Wq = np.asarray(Wq, dtype=np.float32)
    Wk = np.asarray(Wk, dtype=np.float32)
    Wv = np.asarray(Wv, dtype=np.float32)
    Wp = np.asarray(Wp, dtype=np.float32)
    bp = np.asarray(bp, dtype=np.float32)

    if "nc" not in _CACHE:
        _CACHE["nc"] = _build()
    nc = _CACHE["nc"]

    mask = _mask_np()
    WpT = np.ascontiguousarray(Wp.T).astype(bfnp)  # [E(hd), E(n)]
    xTs = [np.ascontiguousarray(x[b].T) for b in range(B)]

    def wslice(W, h0, scale, dt):  # [H,E,D] -> [E, 8*64] col = local head*64+d
        return np.ascontiguousarray(
            (W[h0:h0 + HL] * scale).transpose(1, 0, 2).reshape(E, HL * D)).astype(dt)

    in_maps = []
    for c in range(8):
        b, hhf = c // 2, c % 2
        h0 = hhf * HL
        in_maps.append({
            "xT": xTs[b].astype(bfnp),
            "xT8": xTs[b].astype(fp8np),
            "wq": wslice(Wq, h0, 1.0, bfnp),
            "wk": wslice(Wk, h0, 1.0, bfnp),
            "w8q": wslice(Wq, h0, WSCALE, fp8np),
            "w8k": wslice(Wk, h0, WSCALE, fp8np),
            "wv": wslice(Wv, h0, 1.0, bfnp),
            "wpT": np.ascontiguousarray(WpT[h0 * D:(h0 + HL) * D, :]),
            "maskb": mask,
        })

    res = run_bass_kernel_spmd(nc, in_maps, list(range(8)))
    parts = [np.asarray(res.results[c]["out"], dtype=np.float32) for c in range(8)]
    out = np.stack([parts[2 * b] + parts[2 * b + 1] for b in range(B)], axis=0)
    return (out + bp[None, None, :]).astype(np.float32)


# revision 26
# speedup vs baseline: 1.3146x; 1.0420x over previous
"""Multi-head causal attention (B=4,T=2048,E=1024,H=16,D=64) on 8 TRN2 cores.

Sharding: core c -> batch b=c//2, heads h0=(c%2)*8 (tensor-parallel over head
halves within each batch). Each core computes its 8 heads' attention and a
partial output projection (row-split Wp); host sums the two partials per
batch (+bias).

v3, selective precision:
  - tq tile j=0 (queries 0..511, the largest-|out| rows): fully bf16
    (projection, scores, PV) - zero fp8 noise on early tokens.
  - j>=1: Q/K projections and scores run fp8e4 DoubleRow (2x contraction per
    instr at 0.5 cyc/row); off-diagonal PV runs fp8 DoubleRow over key-chunk
    pairs; diagonal chunk pairs stay bf16 (mask multiply needs exactness).
    Softmax averaging over >=512 keys washes out fp8 quantization noise.
  - exp on ACT, one instruction per chunk-pair over a 2-bank PSUM tile.
  - Q/K fp8 weights pre-scaled x32 (exp scale absorbs /1024); j0 K is
    quantized to fp8 from the bf16 projection, scaled x32 on copy.
Emission interleaves V-proj / next-j projections / prev-j out-proj into the
per-head attention loop so PE has fill work while ACT (the exp bottleneck)
runs.
"""
import sys
import numpy as np

sys.path.insert(0, "/opt/trn_rl_repo")

import ml_dtypes
import concourse.bass as bass
import concourse.bacc as bacc
import concourse.mybir as mybir
from concourse import tile
from concourse.bass_utils import run_bass_kernel_spmd

B, T, E, H, D = 4, 2048, 1024, 16, 64
HL = H // 2      # 8 local heads per core
NP = HL // 2     # 4 head pairs
NJ = T // 512    # 4 tq tiles
NCK = T // 128   # 16 t chunks
NE = E // 128    # 8 e chunks (bf16)
NE2 = E // 256   # 4 e pair-chunks (fp8 DoubleRow)
BF16 = mybir.dt.bfloat16
F32 = mybir.dt.float32
FP8 = mybir.dt.float8e4
bfnp = ml_dtypes.bfloat16
fp8np = ml_dtypes.float8_e4m3
DR = mybir.MatmulPerfMode.DoubleRow
MUL = mybir.AluOpType.mult

WSCALE = 32.0
EXPSCALE8 = 0.125 / (WSCALE * WSCALE)
EXPSCALE = 0.125

_CACHE = {}


def _build():
    nc = bacc.Bacc("TRN2", target_bir_lowering=False)
    xT = nc.declare_dram_parameter("xT", [E, T], BF16, isOutput=False)
    xT8 = nc.declare_dram_parameter("xT8", [E, T], FP8, isOutput=False)
    wq = nc.declare_dram_parameter("wq", [E, HL * D], BF16, isOutput=False)
    wk = nc.declare_dram_parameter("wk", [E, HL * D], BF16, isOutput=False)
    w8q = nc.declare_dram_parameter("w8q", [E, HL * D], FP8, isOutput=False)
    w8k = nc.declare_dram_parameter("w8k", [E, HL * D], FP8, isOutput=False)
    wv = nc.declare_dram_parameter("wv", [E, HL * D], BF16, isOutput=False)
    wpT = nc.declare_dram_parameter("wpT", [HL * D, E], BF16, isOutput=False)
    maskb = nc.declare_dram_parameter("maskb", [128, 128], BF16, isOutput=False)
    out = nc.declare_dram_parameter("out", [T, E], F32, isOutput=True)

    Exp = mybir.ActivationFunctionType.Exp

    with tile.TileContext(nc) as tc:
        with (
            tc.tile_pool(name="persist", bufs=1) as pp,
            tc.tile_pool(name="qtmp", bufs=3) as qtmp,
            tc.tile_pool(name="expp", bufs=4) as expp,
            tc.tile_pool(name="expp8", bufs=4) as expp8,
            tc.tile_pool(name="zpool", bufs=2) as zpool,
            tc.tile_pool(name="outp", bufs=3) as outp,
            tc.tile_pool(name="mm", bufs=2, space=bass.MemorySpace.PSUM) as ps_mm,
            tc.tile_pool(name="sT", bufs=2, space=bass.MemorySpace.PSUM) as ps_sT,
            tc.tile_pool(name="oT", bufs=2, space=bass.MemorySpace.PSUM) as ps_oT,
        ):
            # consolidated tiles: e-chunk as a free dim so each bulk tensor
            # loads with ONE dma_start (HWDGE descriptor gen is 625ns each)
            xT_all = pp.tile([128, NE, T], BF16, tag="xTa", name="xTa")
            x8_all = pp.tile([128, NE2, 2, T - 512], FP8, tag="x8a", name="x8a")
            wq_all = pp.tile([128, NE, HL * D], BF16, tag="bqa", name="bqa")
            wk_all = pp.tile([128, NE, HL * D], BF16, tag="bka", name="bka")
            w8q_all = pp.tile([128, NE2, 2, 512], FP8, tag="wqa", name="wqa")
            w8k_all = pp.tile([128, NE2, 2, 512], FP8, tag="wka", name="wka")
            wv_all = pp.tile([128, NE, HL * D], BF16, tag="wva", name="wva")
            wpT_all = pp.tile([128, NP, E], BF16, tag="wpa", name="wpa")
            xT_sb = [xT_all[:, c] for c in range(NE)]
            x8_sb = [x8_all[:, c] for c in range(NE2)]
            wq_sb = [wq_all[:, c] for c in range(NE)]
            wk_sb = [wk_all[:, c] for c in range(NE)]
            w8q_sb = [w8q_all[:, c] for c in range(NE2)]
            w8k_sb = [w8k_all[:, c] for c in range(NE2)]
            wv_sb = [wv_all[:, c] for c in range(NE)]
            wpT_sb = [wpT_all[:, p] for p in range(NP)]
            mask_sb = pp.tile([128, 128], BF16, tag="mkb", name="mkb")
            v_sb = [pp.tile([128, HL * 65], BF16, tag=f"v{i}", name=f"v{i}") for i in range(NCK)]
            # v8: [128, HL, 2, 128] fp8 - per-head [2,128] block so the
            # DoubleRow Ldweights pair stride is 128 (s3_lw_dual_fp8 wants
            # an aligned pair stride); v at cols 0:64, ones at col 64.
            v8_sb = [pp.tile([128, HL, 2, 128], FP8, tag=f"v8{i}", name=f"v8{i}")
                     for i in range(6)]
            # j0 bf16 q/k: [128, 512]; rows = head (h%2)*64 + d
            qb_sb = [pp.tile([128, 512], BF16, tag=f"qb{p}", name=f"qb{p}") for p in range(NP)]
            kb_sb = [pp.tile([128, 512], BF16, tag=f"kb{p}", name=f"kb{p}") for p in range(NP)]
            # fp8 q/k: [128, 2, T]; head h%2 at partition base 64*(h%2), d as
            # (dhalf -> free slot, 32 -> partition)
            q8_sb = [pp.tile([128, 2, T], FP8, tag=f"q8{p}", name=f"q8{p}") for p in range(NP)]
            k8_sb = [pp.tile([128, 2, T], FP8, tag=f"k8{p}", name=f"k8{p}") for p in range(NP)]
            oT_sb = [[pp.tile([128, 512], BF16, tag=f"o{p}_{j}", name=f"o{p}_{j}") for j in range(NJ)]
                     for p in range(NP)]

            # PE warmup: matmuls on a memset tile so the p-state ramp matures
            # while the first DMAs are in flight (cost model halves the PE
            # clock for the first ~3us of busy time otherwise)
            wu = pp.tile([128, 256], BF16, tag="wu", name="wu")
            nc.gpsimd.memset(wu[:], 0.0)
            for _ in range(20):
                wu_ps = ps_mm.tile([128, 256], F32, tag="mm", name="wups")
                nc.tensor.matmul(wu_ps[:], wu[:, 0:128], wu[:], start=True,
                                 stop=True)

            # ---- DMA in (one dma_start per bulk tensor / wave, first loads
            # spread across engine DGE queues so SP preamble doesn't gate) ----
            def chunked(dram, c):  # [E, X] -> [128, c, X]
                return dram[:].rearrange("(c p) t -> p c t", c=c)

            def paired(dram, col0, col1):  # [E, X] -> [128, NE2, 2, cols]
                return dram[:, col0:col1].rearrange(
                    "(c i p) t -> p c i t", c=NE2, i=2)

            nc.scalar.dma_start(wq_all[:], chunked(wq, NE))
            nc.scalar.dma_start(wk_all[:], chunked(wk, NE))
            nc.sync.dma_start(xT_all[:, :, 0:512],
                              xT[:, 0:512].rearrange("(c p) t -> p c t", c=NE))
            nc.sync.dma_start(wv_all[:], chunked(wv, NE))
            nc.sync.dma_start(mask_sb[:], maskb[:])
            nc.sync.dma_start(w8q_all[:], paired(w8q, 0, 512))
            nc.sync.dma_start(w8k_all[:], paired(w8k, 0, 512))
            for w in range(1, 4):
                nc.sync.dma_start(x8_all[:, :, :, (w - 1) * 512:w * 512],
                                  paired(xT8, w * 512, (w + 1) * 512))
                nc.sync.dma_start(
                    xT_all[:, :, w * 512:(w + 1) * 512],
                    xT[:, w * 512:(w + 1) * 512].rearrange("(c p) t -> p c t", c=NE))
            nc.sync.dma_start(wpT_all[:], chunked(wpT, NP))

            # ---- emission helpers ----
            def fold_dmas(dst, p, jcol, tmp):
                # tmp [128,512] fp8 rows (h, dhalf, 32) -> dst[p] [64h:64h+32, half, cols]
                # plain-slice sources only: a partition-rearranged SBUF source
                # AP breaks subtile dependency tracking (DMA races the write)
                for hh in range(2):
                    for half in range(2):
                        nc.sync.dma_start(
                            dst[p][64 * hh:64 * hh + 32, half, jcol:jcol + 512],
                            tmp[hh * 64 + half * 32:hh * 64 + half * 32 + 32, :])

            def j0_proj(p):
                for wsb, bdst, is_k in ((wq_sb, qb_sb, False), (wk_sb, kb_sb, True)):
                    ps = ps_mm.tile([128, 512], F32, tag="mm", name="mmq")
                    for c in range(NE):
                        nc.tensor.matmul(ps[:], wsb[c][:, p * 128:(p + 1) * 128],
                                         xT_sb[c][:, 0:512],
                                         start=(c == 0), stop=(c == NE - 1))
                    nc.vector.tensor_copy(bdst[p][:], ps[:])
                    if is_k:  # also fold into k8 (x32 to match fp8-side scale)
                        tmp = qtmp.tile([128, 512], FP8, tag="qt", name="qt")
                        nc.vector.tensor_scalar_mul(tmp[:], ps[:], WSCALE)
                        fold_dmas(k8_sb, p, 0, tmp)

            def jn_proj(j, p, w8_sb, dst):
                ps = ps_mm.tile([128, 512], F32, tag="mm", name="mmq")
                for c2 in range(NE2):
                    nc.tensor.matmul(
                        ps[:], w8_sb[c2][:, :, p * 128:(p + 1) * 128],
                        x8_sb[c2][:, :, (j - 1) * 512:j * 512],
                        start=(c2 == 0), stop=(c2 == NE2 - 1), perf_mode=DR)
                tmp = qtmp.tile([128, 512], FP8, tag="qt", name="qt")
                nc.vector.tensor_copy(tmp[:], ps[:])
                fold_dmas(dst, p, j * 512, tmp)

            def v_proj(i):
                ps = ps_mm.tile([128, HL * D], F32, tag="mm", name="mmv")
                for c in range(NE):
                    nc.tensor.matmul(ps[:], xT_sb[c][:, i * 128:(i + 1) * 128],
                                     wv_sb[c][:], start=(c == 0), stop=(c == NE - 1))
                v3 = v_sb[i][:].rearrange("p (h d) -> p h d", d=65)
                nc.gpsimd.memset(v_sb[i][:], 1.0)
                nc.vector.tensor_copy(
                    v3[:, :, 0:64], ps[:].rearrange("p (h d) -> p h d", d=64))
                if i < 12:
                    cp, ci = i // 2, i % 2
                    if ci == 0:
                        nc.gpsimd.memset(v8_sb[cp][:], 1.0)
                    nc.vector.tensor_copy(
                        v8_sb[cp][:, :, ci, 0:64],
                        ps[:].rearrange("p (h d) -> p h d", d=64))

            def attn(j, h):
                p, hh = h // 2, h % 2
                rb = 64 * hh
                oT_ps = ps_oT.tile([65, 512], F32, tag="oT", name="oTps")
                nblk = 4 * (j + 1)
                npair = nblk // 2
                for cp in range(npair):
                    offdiag = j > 0 and cp < 2 * j
                    cw = []
                    for ci in range(2):
                        c = 2 * cp + ci
                        r = max(0, (c - 4 * j) * 128)
                        cw.append((c, r, 512 - r))
                    sT = ps_sT.tile([128, 2, 512], F32, tag="sT", name="sTps")
                    for ci, (c, r, w) in enumerate(cw):
                        if j == 0:
                            nc.tensor.matmul(
                                sT[:, ci, 0:w],
                                kb_sb[p][rb:rb + 64, c * 128:(c + 1) * 128],
                                qb_sb[p][rb:rb + 64, r:512],
                                start=True, stop=True)
                        else:
                            nc.tensor.matmul(
                                sT[:, ci, 0:w],
                                k8_sb[p][rb:rb + 32, :, c * 128:(c + 1) * 128],
                                q8_sb[p][rb:rb + 32, :, j * 512 + r:(j + 1) * 512],
                                start=True, stop=True, perf_mode=DR)
                    scale = EXPSCALE if j == 0 else EXPSCALE8
                    if offdiag:
                        et8 = expp8.tile([128, 2, 512], FP8, tag="e8", name="e8")
                        nc.scalar.activation(et8[:], sT[:], Exp, scale=scale)
                        nc.tensor.matmul(
                            oT_ps[:, 0:512],
                            v8_sb[cp][:, h, :, 0:65],
                            et8[:],
                            start=(cp == 0), stop=False,
                            perf_mode=DR, skip_group_check=True)
                        continue
                    et = expp.tile([128, 2, 512], BF16, tag="et", name="et")
                    w0, w1 = cw[0][2], cw[1][2]
                    if w0 == 512:       # diag pair 1: widths 512, 384
                        sflat = sT[:].rearrange("p a b -> p (a b)")
                        eflat = et[:].rearrange("p a b -> p (a b)")
                        nc.scalar.activation(eflat[:, 0:896], sflat[:, 0:896],
                                             Exp, scale=scale)
                    else:               # diag pair 2: widths 256, 128
                        nc.scalar.activation(et[:, :, 0:128], sT[:, :, 0:128],
                                             Exp, scale=scale)
                        nc.scalar.activation(et[:, 0, 128:256], sT[:, 0, 128:256],
                                             Exp, scale=scale)
                    for ci, (c, r, w) in enumerate(cw):
                        nc.vector.tensor_mul(et[:, ci, 0:128],
                                             et[:, ci, 0:128], mask_sb[:])
                    v3s = [v_sb[c][:].rearrange("p (h d) -> p h d", d=65)
                           for c, _, _ in cw]
                    for ci, (c, r, w) in enumerate(cw):
                        nc.tensor.matmul(oT_ps[:, r:512],
                                         v3s[ci][:, h, :],
                                         et[:, ci, 0:w],
                                         start=(c == 0), stop=(c == nblk - 1),
                                         skip_group_check=True)
                zi = zpool.tile([1, 512], F32, tag="zi", name="zi")
                nc.vector.reciprocal(zi[:], oT_ps[64:65, :])
                zb = zpool.tile([64, 512], F32, tag="zb", name="zb")
                nc.gpsimd.partition_broadcast(zb[:], zi[:])
                nc.vector.tensor_mul(oT_sb[p][j][64 * hh:64 * hh + 64, :],
                                     oT_ps[0:64, :], zb[:])

            def out_proj(j, t):
                ob = outp.tile([128, E], F32, tag="ob", name="ob")
                for n in range(2):
                    ps = ps_mm.tile([128, 512], F32, tag="mm", name="mmo")
                    for p in range(NP):
                        nc.tensor.matmul(
                            ps[:],
                            oT_sb[p][j][:, (t % 4) * 128:(t % 4 + 1) * 128],
                            wpT_sb[p][:, n * 512:(n + 1) * 512],
                            start=(p == 0), stop=(p == NP - 1))
                    nc.vector.tensor_copy(ob[:, n * 512:(n + 1) * 512], ps[:])
                    nc.sync.dma_start(
                        out[t * 128:(t + 1) * 128, n * 512:(n + 1) * 512],
                        ob[:, n * 512:(n + 1) * 512])

            # ---- emission schedule ----
            # start attention as early as possible: only pair 0's bf16
            # projection and V chunks 0..3 gate attn(0,0); the rest of the
            # j0 projections and all later work rides in the filler slots.
            j0_proj(0)
            for i in range(4):
                v_proj(i)

            # filler queues per j-loop (popped between heads to keep PE busy
            # while ACT works); each phase's filler PE budget roughly matches
            # that phase's ACT time minus its attention PE time
            fillers = {
                0: ([("j0p", 1), ("pq", 1, 0), ("pk", 1, 0), ("j0p", 2),
                     ("j0p", 3), ("pq", 1, 1), ("pk", 1, 1), ("v", 4),
                     ("pq", 1, 2), ("pk", 1, 2), ("v", 5),
                     ("pq", 1, 3), ("pk", 1, 3), ("v", 6), ("v", 7)]),
                1: ([("pq", 2, 0), ("pk", 2, 0), ("v", 8), ("o", 0, 0),
                     ("pq", 2, 1), ("pk", 2, 1), ("v", 9), ("o", 0, 1),
                     ("pq", 2, 2), ("pk", 2, 2), ("v", 10), ("o", 0, 2),
                     ("pq", 2, 3), ("pk", 2, 3), ("v", 11), ("o", 0, 3)]),
                2: ([("pq", 3, 0), ("pk", 3, 0), ("v", 12), ("o", 1, 0),
                     ("pq", 3, 1), ("pk", 3, 1), ("v", 13), ("o", 1, 1),
                     ("pq", 3, 2), ("pk", 3, 2), ("v", 14), ("o", 1, 2),
                     ("pq", 3, 3), ("pk", 3, 3), ("v", 15), ("o", 1, 3)]),
                3: ([("o", 2, 0), ("o", 2, 1), ("o", 2, 2), ("o", 2, 3)]),
            }

            def run_filler(f):
                if f[0] == "v":
                    v_proj(f[1])
                elif f[0] == "j0p":
                    j0_proj(f[1])
                elif f[0] == "pq":
                    jn_proj(f[1], f[2], w8q_sb, q8_sb)
                elif f[0] == "pk":
                    jn_proj(f[1], f[2], w8k_sb, k8_sb)
                elif f[0] == "o":
                    out_proj(f[1], f[1] * 4 + f[2])

            for j in range(NJ):
                q = list(fillers[j])
                per = (len(q) + HL - 1) // HL
                for h in range(HL):
                    attn(j, h)
                    for _ in range(per):
                        if q:
                            run_filler(q.pop(0))
                while q:
                    run_filler(q.pop(0))
            for t in range(12, 16):
                out_proj(3, t)

    nc.compile()
    return nc


def _mask_np():
    f = np.arange(128)[None, :]
    p = np.arange(128)[:, None]
    return (f >= p).astype(bfnp)


def kernel(x, Wq, Wk, Wv, Wp, bp):
    x = np.asarray(x, dtype=np.float32)
    Wq = np.asarray(Wq, dtype=np.float32)
    Wk = np.asarray(Wk, dtype=np.float32)
    Wv = np.asarray(Wv, dtype=np.float32)
    Wp = np.asarray(Wp, dtype=np.float32)
    bp = np.asarray(bp, dtype=np.float32)

    if "nc" not in _CACHE:
        _CACHE["nc"] = _build()
    nc = _CACHE["nc"]

    mask = _mask_np()
    WpT = np.ascontiguousarray(Wp.T).astype(bfnp)  # [E(hd), E(n)]
    xTs = [np.ascontiguousarray(x[b].T) for b in range(B)]

    def wslice(W, h0, scale, dt):  # [H,E,D] -> [E, 8*64] col = local head*64+d
        return np.ascontiguousarray(
            (W[h0:h0 + HL] * scale).transpose(1, 0, 2).reshape(E, HL * D)).astype(dt)

    in_maps = []
    for c in range(8):
        b, hhf = c // 2, c % 2
        h0 = hhf * HL
        in_maps.append({
            "xT": xTs[b].astype(bfnp),
            "xT8": xTs[b].astype(fp8np),
            "wq": wslice(Wq, h0, 1.0, bfnp),
            "wk": wslice(Wk, h0, 1.0, bfnp),
            "w8q": wslice(Wq, h0, WSCALE, fp8np),
            "w8k": wslice(Wk, h0, WSCALE, fp8np),
            "wv": wslice(Wv, h0, 1.0, bfnp),
            "wpT": np.ascontiguousarray(WpT[h0 * D:(h0 + HL) * D, :]),
            "maskb": mask,
        })

    res = run_bass_kernel_spmd(nc, in_maps, list(range(8)))
    parts = [np.asarray(res.results[c]["out"], dtype=np.float32) for c in range(8)]
    out = np.stack([parts[2 * b] + parts[2 * b + 1] for b in range(B)], axis=0)
    return (out + bp[None, None, :]).astype(np.float32)
